# revision 40
# baseline (speedup 1.0000x reference)
"""Trainium2 Bass kernel for a 16-head causal MHA layer with relative-position
bias (B=2, S=2048, D=1024, H=16, HD=64), distributed over 8 NeuronCores.

Sharding: tensor parallel over heads — core c computes heads {2c, 2c+1} for
both batches.  The output projection is sharded over its input dim, so each
core returns a partial (B, S, D) output; the partials are summed on the host
(plus proj_b).

Per-core device pipeline (matmul operands in fp16):
  1. QKV projections in transposed layout: QT/KT/VT (128=2*HD, 2048) from
     per-k-chunk XT tiles x weight-slice chunks.  K' = K + rpr[positions]
     folded into the same PSUM accumulation group via a stacked-identity
     matmul, so the eviction is a plain copy.
  2. Attention per head in S^T layout: scores S^T(j,i) = K'^T-slice . Q-slice
     (both heads packed in one PE pass via tile_position row tiling, which
     runs concurrently on HW), exp on the scalar engine (scale folded in;
     the natural_log_exp_and_others act table is pre-loaded once so no
     per-use table reloads occur), causal masking via a 0/1-tril multiply
     on the exp OUTPUT, then OT_aug(65, i) += V_aug(j, 65)^T . P^T with a
     ones-column producing the softmax denominators for free.  Fully-masked
     j-blocks are skipped, fully-masked lead columns of diagonal blocks are
     not computed.
  3. Normalize (staged into the next block so no engine queue stalls):
     1/denom via a single custom-DVE reciprocal_approx_fast (fp32, via a
     small SBUF staging copy — the custom op mis-reads PSUM), partition-
     broadcast on gpsimd, then a DVE multiply reading OT_aug directly from
     PSUM.
  4. Output projection: y_partial(s, e) += OT2^T . pwT -> PSUM -> DVE/ACT
     eviction into a 4-block staging tile, one DMA per (b, it).

Scheduling: each batch's attention phase overlaps the OTHER batch's
QKV/V-transpose filler units on the in-order PE queue, interleaved in
next-phase consumption order; x loads are issued a phase ahead (per-chunk
DMAs/tiles); constants load in 4 consolidated DMAs (each dma_start costs
~650ns of SP-sequencer issue time).
"""

import sys

import numpy as np

try:
    import concourse.bass as bass  # noqa: F401
except ImportError:
    sys.path.insert(0, "/opt/trn_rl_repo")

import concourse.bass as bass
import concourse.mybir as mybir
import concourse.tile as tile
from concourse import bacc
from concourse.bass_utils import run_bass_kernel_spmd

B, S, D, H = 2, 2048, 1024, 16
HD = D // H  # 64
SCALE = HD**-0.5
N_CORES = 8
HPC = H // N_CORES  # heads per core = 2
DL = HPC * HD  # local head channels = 128
NJ = S // 128  # 16 j-chunks of 128
NI = S // 512  # 4 i-blocks of 512
KC = D // 128  # 8 contraction chunks of 128

F32 = mybir.dt.float32
F32R = mybir.dt.float32r
F16 = mybir.dt.float16

import os

# tunables
TRIL_ENGINE = os.environ.get("K_TRIL", "vector")  # causal 0/1 multiply engine
Y_TO_ACT = int(os.environ.get("K_YACT", "1"))  # y evictions to scalar, of 4
AV_SPLIT = bool(int(os.environ.get("K_AVSPLIT", "1")))  # defer masked AV
SC_TRIM = True  # skip fully-masked lead columns in the scores matmul
NORM_BC = os.environ.get("K_NORM", "gpsimd")  # "gpsimd" | "pe"
SC_SPLIT = bool(int(os.environ.get("K_SCSPLIT", "0")))  # per-head score tiles
PT_BUFS = int(os.environ.get("K_PTBUFS", "3"))  # exp-output ring depth
# (either way the multiply reads only one PSUM operand — a HW requirement)
SKIP_XT = bool(int(os.environ.get("K_SKIP_XT", "0")))  # ablation: no x loads
SKIP_Y = bool(int(os.environ.get("K_SKIP_Y", "0")))  # ablation: no y stores
HALF_EXP = bool(int(os.environ.get("K_HALF_EXP", "0")))  # ablation probe
HALF_AV = bool(int(os.environ.get("K_HALF_AV", "0")))  # ablation probe
NO_NORM = bool(int(os.environ.get("K_NO_NORM", "0")))  # ablation probe
NO_YEV = bool(int(os.environ.get("K_NO_YEV", "0")))  # ablation probe
NOREC = bool(int(os.environ.get("K_NOREC", "0")))  # ablation probe
NO_TRIL = bool(int(os.environ.get("K_NO_TRIL", "0")))  # ablation probe
PREMASK = bool(int(os.environ.get("K_PREMASK", "0")))  # -60000 add pre-exp
INLINE_NORM = bool(int(os.environ.get("K_INORM", "0")))  # rec/bc/mul inline
RPR_DVE = bool(int(os.environ.get("K_RPR_DVE", "0")))  # rpr add on DVE
VA_DMA = bool(int(os.environ.get("K_VA_DMA", "0")))  # V transpose via DMA
NORM_REC = os.environ.get("K_REC", "act")  # "act": 1/x = exp(-ln x); "dve"
NO_ATT = bool(int(os.environ.get("K_NO_ATT", "0")))  # ablation probe
NO_PROJ = bool(int(os.environ.get("K_NO_PROJ", "0")))  # ablation probe
SCHED = os.environ.get("K_SCHED", "v3")  # v2: end-of-phase va; v3: inline
UB = [int(x) for x in os.environ.get("K_UB", "0,2,5,10,16").split(",")]
QEV_ACT = bool(int(os.environ.get("K_QEV_ACT", "0")))  # qkv evict on ACT

_BUILD_CACHE: dict = {}


def _emit(nc, tc, t, mode, niter):
    xt = t["xt"].ap()  # (B, 128, 8*2048) f16 (see _prep_inputs layout)
    wall = t["wall"].ap()  # (128, 3*KC*DL) f16: col (p*KC+k)*DL+c =
    #   w_p^T[k*128+r, c] — all three projection weights, one DMA
    pwT = t["pwT"].ap()  # (DL, D) f16
    rpr2 = t["rpr2T"].ap()  # (128, S) f16 : b-th 64 rows = rprT for batch b
    misc = t["misc"].ap()  # (128, 3*128+1) f16: [tril01 | ident | i2 | ones]
    y = t["y"].ap()  # (B, NI, 128, 4*1024) f16
    maskT = t["maskT"].ap() if "maskT" in t else None  # (S, S) f32

    # Pre-load the natural_log_exp_and_others act-function set (id 6): it
    # serves Exp, Ln AND Copy, so the compiler's table-load pass finds every
    # activation already satisfiable and inserts no per-use reloads (the
    # greedy per-func choice would otherwise toggle exp_and_others <->
    # natural_log around each Ln pair at 1.28us per reload).
    nc.scalar.add_instruction(mybir.InstLoadActFuncSet(
        act_func_set_id=6, name=nc.get_next_instruction_name(),
        engine=mybir.EngineType.Activation, ins=[], outs=[]))

    ctxs = [
        tc.tile_pool(name="consts", bufs=1),
        tc.tile_pool(name="xt", bufs=1),
        tc.tile_pool(name="qkv", bufs=1),
        tc.tile_pool(name="va", bufs=1),
        tc.tile_pool(name="pt", bufs=PT_BUFS),
        tc.tile_pool(name="sm", bufs=2),
        tc.tile_pool(name="ysb", bufs=2),
        tc.tile_pool(name="ps_mm", bufs=2, space="PSUM"),
        tc.tile_pool(name="ps_sc", bufs=4 if SC_SPLIT else 2, space="PSUM"),
        tc.tile_pool(name="ps_ot", bufs=2, space="PSUM"),
    ]
    if maskT is not None:
        ctxs.append(tc.tile_pool(name="mk", bufs=4))
    pools = [c.__enter__() for c in ctxs]
    (consts, xtp, qkvp, vap, ptp, smp, ysbp, ps_mm, ps_sc, ps_ot) = pools[:10]
    mkp = pools[10] if maskT is not None else None

    # --- persistent constants (loaded once, outside the timing loop).
    # Consolidated into 3 DMAs: each dma_start costs ~650ns of SP-sequencer
    # issue time, so 31 separate const loads would stall the prologue ~20us.
    wall_t = consts.tile([128, 3 * KC * DL], F16, tag="wall")
    nc.sync.dma_start(wall_t[:], wall)
    w_t = [[wall_t[:, (p * KC + k) * DL:(p * KC + k + 1) * DL]
            for k in range(KC)] for p in range(3)]
    pw_t = consts.tile([DL, D], F16, tag="pw")
    nc.sync.dma_start(pw_t[:], pwT)
    rpr_t = consts.tile([128, S], F16, tag="rpr")
    nc.sync.dma_start(rpr_t[:], rpr2)
    misc_t = consts.tile([128, 4 * 128 + 1 + HD], F16, tag="misc")
    nc.sync.dma_start(misc_t[:], misc)
    tril_t = misc_t[:, 0:128]
    id_t = misc_t[:, 128:256]
    i2_t = misc_t[:, 256:384]
    ones_t = misc_t[:, 384:385]
    ones1f_t = consts.tile([1, HD], F32, tag="ones1f")
    nc.vector.tensor_copy(ones1f_t[:], misc_t[0:1, 385:385 + HD])
    trimask_t = misc_t[:, 385 + HD:385 + HD + 128]  # strict-upper -60000

    # persistent V_aug tiles with the ones columns preset once:
    # layout [v_h0(0:64) | 1(64) | v_h1(65:129) | 1(129)]
    va_t = {(b, j): vap.tile([128, 2 * (HD + 1)], F16, tag=f"va{b}_{j}",
                             name=f"va{b}_{j}")
            for b in range(B) for j in range(NJ)}
    for (b, j), va in va_t.items():
        nc.vector.tensor_copy(va[:, HD:HD + 1], ones_t[:])
        nc.vector.tensor_copy(va[:, 2 * HD + 1:2 * HD + 2], ones_t[:])

    # persistent tiles referenced across loop iterations (bufs=1 tags).
    # x is held as one tile PER k-chunk so each chunk's DMA -> matmul
    # dependency is tracked separately (a single big tile would make the
    # first matmul wait for the whole 4MB load).
    xts = {(b, k): xtp.tile([128, S], F16, tag=f"xt{b}_{k}",
                            name=f"xt{b}_{k}")
           for b in range(B) for k in range(KC)}
    qkv = {b: (qkvp.tile([128, S], F16, tag=f"qt{b}", name=f"qt{b}"),
               qkvp.tile([128, S], F16, tag=f"kt{b}", name=f"kt{b}"),
               qkvp.tile([128, S], F16, tag=f"vt{b}", name=f"vt{b}"))
           for b in range(B)}
    ot2s = {b: qkvp.tile([128, S], F16, tag=f"ot2_{b}", name=f"ot2_{b}")
            for b in range(B)}

    def make_body():

        def emit_loads(b):
            if not SKIP_XT:
                # per-k-chunk DMAs so the first QKV matmul (which reads only
                # chunk 0) can start ~1/8 of the way into the transfer
                for k in range(KC):
                    nc.sync.dma_start(xts[(b, k)][:],
                                      xt[b, :, k * S:(k + 1) * S])

        def emit_qkv_group(b, g):
            # g in 0..11: projection p = g // NI, 512-col block sb = g % NI
            emit_qkv_psb(b, g // NI, g % NI)

        def emit_qkv_psb(b, p, sb):
            dst = qkv[b][p]
            ps = ps_mm.tile([128, 512], F32, tag="mm")
            for k in range(KC):
                nc.tensor.matmul(
                    ps[:], w_t[p][k][:],
                    xts[(b, k)][:, sb * 512:(sb + 1) * 512],
                    start=(k == 0),
                    stop=((p != 1 or RPR_DVE) and k == KC - 1))
            sl = slice(sb * 512, (sb + 1) * 512)
            if p == 1 and not RPR_DVE:
                # K' = K + rpr via a rank-64 stacked-identity matmul in the
                # same accumulation group (same bias for both head halves)
                nc.tensor.matmul(ps[:], i2_t[64 * b:64 * b + 64, :],
                                 rpr_t[64 * b:64 * b + 64, sl],
                                 start=False, stop=True)
            if p == 1 and RPR_DVE:
                # fold the rpr bias during eviction on the DVE instead of
                # spending PE cycles: one add per 64-row head half
                for hh in range(2):
                    nc.vector.tensor_add(
                        dst[64 * hh:64 * hh + 64, sl],
                        ps[64 * hh:64 * hh + 64, :],
                        rpr_t[64 * b:64 * b + 64, sl])
            elif QEV_ACT:
                nc.scalar.activation(dst[:, sl], ps[:],
                                     mybir.ActivationFunctionType.Copy)
            else:
                nc.vector.tensor_copy(dst[:, sl], ps[:])

        def emit_va(b):
            vt = qkv[b][2]
            for j in range(NJ):
                tp = ps_mm.tile([128, 128], F16, tag="mm", name="tp")
                nc.tensor.transpose(tp[:], vt[:, j * 128:(j + 1) * 128], id_t[:])
                va = va_t[(b, j)]
                nc.vector.tensor_copy(
                    va[:].rearrange("p (g x) -> p g x", g=2)[:, :, 0:HD],
                    tp[:].rearrange("p (g x) -> p g x", g=2))

        def emit_proj_blocks(b, sts, yt):
            if NO_PROJ:
                return None
            it = sts[0] // 4
            if yt is None:
                yt = ysbp.tile([128, 4 * D], F16, tag="y")
            last = (sts[-1] + 1) % 4 == 0
            for stq in sts:
                ssl = slice(stq * 128, (stq + 1) * 128)
                c = stq % 4
                for eb in range(D // 512):
                    pp = ps_mm.tile([128, 512], F32, tag="mm", name="pp")
                    nc.tensor.matmul(
                        pp[:], ot2s[b][:, ssl],
                        pw_t[:, eb * 512:(eb + 1) * 512],
                        start=True, stop=True)
                    ysl = slice(c * D + eb * 512, c * D + (eb + 1) * 512)
                    if NO_YEV:
                        continue
                    if (2 * c + eb) % 4 < Y_TO_ACT:
                        nc.scalar.activation(
                            yt[:, ysl], pp[:],
                            mybir.ActivationFunctionType.Copy)
                    else:
                        nc.vector.tensor_copy(yt[:, ysl], pp[:])
            if last and not (SKIP_Y or NO_YEV):
                nc.sync.dma_start(y[b, it], yt[:])
            return yt

        def emit_att_stub(b, it):
            isl = slice(it * 512, (it + 1) * 512)
            nc.vector.tensor_copy(ot2s[b][:, isl], rpr_t[:, 0:512])
            emit_proj_blocks(b, range(4 * it, 4 * it + 4), None)

        pending_stages = []

        def pump(n=1):
            for _ in range(n):
                if pending_stages:
                    pending_stages.pop(0)()

        def flush_finish():
            pump(len(pending_stages))

        def emit_att_it(b, it):
            if NO_ATT:
                emit_att_stub(b, it)
                return
            # scores -> exp -> AV for one 512-wide query block, software-
            # pipelined two j-chunks deep so the in-order PE queue is not
            # gated on the ACT exp latency each chunk.  The normalize +
            # projection of each block is DEFERRED into the next block's
            # scores phase so the PE never stalls on the norm chain.
            qt, kt, _ = qkv[b]
            isl = slice(it * 512, (it + 1) * 512)
            jhi = (4 * it + 3) if mode == "causal" else (NJ - 1)
            otp = [ps_ot.tile([HD + 1, 512], F32, tag="ot", name=f"ot{_h}")
                   for _h in range(HPC)]

            def emit_scores(j):
                jsl = slice(j * 128, (j + 1) * 128)
                dc = max(0, (j - 4 * it) * 128) if mode == "causal" else 0
                dct = dc if SC_TRIM else 0
                if SC_SPLIT:
                    tiles = [ps_sc.tile([128, 512], F32, tag="sc",
                                        name=f"sc{_h}") for _h in range(HPC)]
                    views = [lambda lo, hi, t=t: t[:, lo:hi] for t in tiles]
                    whole = None
                else:
                    sc2 = ps_sc.tile([128, 2 * 512], F32, tag="sc",
                                     name="sc2")
                    views = [
                        lambda lo, hi, h=h: sc2[:, h * 512 + lo:h * 512 + hi]
                        for h in range(HPC)]
                    whole = sc2
                for h in range(HPC):
                    hsl = slice(h * HD, (h + 1) * HD)
                    nc.tensor.matmul(
                        views[h](dct, 512), kt[hsl, jsl],
                        qt[hsl, isl.start + dct:isl.stop], start=True,
                        stop=True, tile_position=(h * HD, 0))
                if maskT is not None:
                    mkt = mkp.tile([128, 512], F32, tag="mk")
                    nc.sync.dma_start(mkt[:], maskT[jsl, isl])
                    for h in range(HPC):
                        nc.vector.tensor_add(
                            views[h](0, 512), views[h](0, 512), mkt[:])
                if PREMASK and mode == "causal" and j >= 4 * it:
                    # additive causal mask on the diagonal 128-block of the
                    # scores PSUM, BEFORE the exp (same semantics as the
                    # reference).  Runs ~2 j-chunks ahead of the exp, so the
                    # DVE hop is off the exp->AV critical path (unlike the
                    # 0/1-tril multiply on the exp output it replaces).
                    for h in range(HPC):
                        nc.vector.tensor_add(
                            views[h](dc, dc + 128), views[h](dc, dc + 128),
                            trimask_t)
                return views, whole

            def emit_exp_av(j, sc):
                views, whole = sc
                dc = max(0, (j - 4 * it) * 128) if mode == "causal" else 0
                diag = mode == "causal" and j >= 4 * it
                pt2 = ptp.tile([128, 2 * 512], F16, tag="pt", name="pt2")
                if HALF_EXP:
                    # timing probe: exp only head 0; head 1 reads h0's P
                    nc.scalar.activation(
                        pt2[:, dc:512], views[0](dc, 512),
                        mybir.ActivationFunctionType.Exp, scale=SCALE)
                elif dc == 0 and whole is not None:
                    nc.scalar.activation(
                        pt2[:], whole[:],
                        mybir.ActivationFunctionType.Exp, scale=SCALE)
                else:
                    for h in range(HPC):
                        nc.scalar.activation(
                            pt2[:, h * 512 + dc:(h + 1) * 512],
                            views[h](dc, 512),
                            mybir.ActivationFunctionType.Exp, scale=SCALE)
                va = va_t[(b, j)]

                def ptb(h):
                    return 0 if HALF_EXP else h * 512

                class _NopEng:
                    def tensor_mul(self, *a, **k):
                        pass
                eng = (_NopEng() if (NO_TRIL or PREMASK) else
                       nc.gpsimd if TRIL_ENGINE == "pool" else nc.vector)
                if diag and AV_SPLIT and not PREMASK and j > 0:
                    # the diagonal 128x128 block needs the 0/1-tril multiply
                    # on the exp output; AV over the unmasked columns runs
                    # immediately, the masked-block AV trails off-path (the
                    # accumulation order into otp does not matter)
                    for h in range(HPC):
                        if dc + 128 < 512:
                            nc.tensor.matmul(
                                otp[h][:, dc + 128:512],
                                va[:, h * (HD + 1):(h + 1) * (HD + 1)],
                                pt2[:, ptb(h) + dc + 128:ptb(h) + 512],
                                start=(j == 0), stop=(j == jhi),
                                skip_group_check=True)
                        if not (HALF_EXP and h):
                            eng.tensor_mul(
                                pt2[:, ptb(h) + dc:ptb(h) + dc + 128],
                                pt2[:, ptb(h) + dc:ptb(h) + dc + 128],
                                tril_t[:])
                        nc.tensor.matmul(
                            otp[h][:, dc:dc + 128],
                            va[:, h * (HD + 1):(h + 1) * (HD + 1)],
                            pt2[:, ptb(h) + dc:ptb(h) + dc + 128],
                            start=(j == 0), stop=(j == jhi),
                            skip_group_check=True)
                else:
                    if diag:
                        for h in range(HPC):
                            if HALF_EXP and h:
                                continue
                            eng.tensor_mul(
                                pt2[:, ptb(h) + dc:ptb(h) + dc + 128],
                                pt2[:, ptb(h) + dc:ptb(h) + dc + 128],
                                tril_t[:])
                    for h in range(1 if HALF_AV else HPC):
                        nc.tensor.matmul(
                            otp[h][:, dc:512],
                            va[:, h * (HD + 1):(h + 1) * (HD + 1)],
                            pt2[:, ptb(h) + dc:ptb(h) + 512],
                            start=(j == 0), stop=(j == jhi),
                            skip_group_check=True)

            pend = [(0, emit_scores(0))]
            if jhi >= 1:
                pend.append((1, emit_scores(1)))
            pump()
            for j in range(2, jhi + 1):
                pend.append((j, emit_scores(j)))
                pump()
                jd, scd = pend.pop(0)
                emit_exp_av(jd, scd)
            flush_finish()
            for jd, scd in pend:
                emit_exp_av(jd, scd)

            # the normalize + projection chain is emitted as STAGES pumped
            # one scores-chunk apart during the NEXT block, so each hop's
            # input is complete before its (in-order) engine reaches it:
            # recs -> broadcasts -> multiplies -> proj(st01) -> proj(st23).
            # The OT_aug PSUM tiles are read directly by the rec + mul
            # stages (no SBUF eviction); the ps_ot ring (bufs=2) keeps the
            # next block in the other slot, and stage_mul retires this slot
            # well before block it+2 needs it.
            st = {}

            def stage_rec():
                st["rec"] = []
                for h in range(HPC):
                    op = otp[0] if HALF_AV else otp[h]
                    dn = smp.tile([1, 512], F32, tag="dn")
                    rec = smp.tile([1, 512], F32, tag="rec")
                    if not NO_NORM:
                        # ~18-bit 1/x in a single custom-DVE op; keeps the
                        # scalar engine free for the softmax exps.  The
                        # denom row goes through SBUF first — the custom op
                        # mis-reads PSUM sources.
                        nc.vector.tensor_copy(dn[:], op[HD:HD + 1, :])
                        if NOREC:
                            nc.vector.tensor_copy(rec[:], dn[:])
                        else:
                            nc.vector.reciprocal_approx_fast(rec[:], dn[:])
                    st["rec"].append(rec)

            def stage_bc():
                st["bc"] = []
                for h in range(HPC):
                    if NORM_BC == "bcast":
                        st["bc"].append(None)
                        continue
                    bc = smp.tile([HD, 512],
                                  F16 if NORM_BC == "pe" else F32, tag="bc")
                    if not NO_NORM:
                        if NORM_BC == "pe":
                            # rank-1 outer product ones^T x rec on the PE
                            # (f32r views: 1 cyc/row); gpsimd
                            # partition_broadcast can be slow on HW
                            bcp = ps_mm.tile([HD, 512], F32, tag="mm",
                                             name="bcp")
                            nc.tensor.matmul(bcp[:],
                                             ones1f_t[:].bitcast(F32R),
                                             st["rec"][h][:].bitcast(F32R),
                                             start=True, stop=True)
                            nc.vector.tensor_copy(bc[:], bcp[:])
                        else:
                            nc.gpsimd.partition_broadcast(
                                bc[:], st["rec"][h][:])
                    st["bc"].append(bc)

            def stage_mul():
                for h in range(HPC):
                    op = otp[0] if HALF_AV else otp[h]
                    if NO_NORM:
                        src = rpr_t[0:HD, 0:512]
                    elif NORM_BC == "bcast":
                        src = st["rec"][h][:].broadcast_to([HD, 512])
                    else:
                        src = st["bc"][h][:]
                    nc.vector.tensor_mul(
                        ot2s[b][h * HD:(h + 1) * HD, isl], op[0:HD, :],
                        src)

            def stage_proj01():
                st["yt"] = emit_proj_blocks(b, range(4 * it, 4 * it + 2),
                                            None)

            def stage_proj23():
                emit_proj_blocks(b, range(4 * it + 2, 4 * it + 4), st["yt"])

            if INLINE_NORM:
                # rec/bc/mul touch only DVE+Pool; emitting them inline at
                # block end starts the chain earlier and frees the ot PSUM
                # ring sooner.  Only the PE-bound proj stages stay deferred.
                stage_rec()
                stage_bc()
                stage_mul()
                pending_stages.extend(
                    [lambda: None, stage_proj01, stage_proj23])
            else:
                pending_stages.extend(
                    [stage_rec, stage_bc, stage_mul, stage_proj01,
                     stage_proj23])

        def emit_va_set(b, sb):
            # V^T -> va transposes for j-chunks 4sb..4sb+3 (follows the
            # (2, sb) projection group that produced those vt columns)
            vt = qkv[b][2]
            for j in range(4 * sb, 4 * sb + 4):
                va = va_t[(b, j)]
                if VA_DMA:
                    # DMA xbar transpose to a 2D scratch (the xbar can't
                    # target the interleaved va view), then DVE-interleave;
                    # saves the PE pass + PSUM ring slot
                    tps = smp.tile([128, 128], F16, tag="vtp", name="vtp")
                    nc.sync.dma_start_transpose(
                        tps[:], vt[:, j * 128:(j + 1) * 128])
                    nc.vector.tensor_copy(
                        va[:].rearrange("p (g x) -> p g x", g=2)[:, :, 0:HD],
                        tps[:].rearrange("p (g x) -> p g x", g=2))
                    continue
                tp = ps_mm.tile([128, 128], F16, tag="mm", name="tp")
                nc.tensor.transpose(tp[:], vt[:, j * 128:(j + 1) * 128],
                                    id_t[:])
                nc.vector.tensor_copy(
                    va[:].rearrange("p (g x) -> p g x", g=2)[:, :, 0:HD],
                    tp[:].rearrange("p (g x) -> p g x", g=2))

        def fill_units(b):
            # filler work in next-phase consumption order: (q,k,v,va) per
            # 512-col block, so att(b, it) of the next phase finds qt/kt/va
            # for its own window already complete, and the va copies are
            # spread across the phase instead of bursting at its end
            units = []
            for sb in range(NI):
                for p in range(3):
                    units.append(
                        lambda b=b, p=p, sb=sb: emit_qkv_psb(b, p, sb))
                units.append(lambda b=b, sb=sb: emit_va_set(b, sb))
            return units

        def prologue():
            # one-time pipeline fill: batch 0's QKV ahead of the loop, and
            # batch 1's x staged so the first body's fillers don't wait.
            # loads(1) is issued AFTER the QKV emission: the 8 DMA queues
            # share bandwidth, so batching both 4MB loads up front would
            # delay batch 0's first chunk (and the whole pipeline) 2x.
            emit_loads(0)
            for g in range(3 * NI):
                emit_qkv_group(0, g)
            emit_loads(1)
            emit_va(0)

        def body(_iv=None):
            # Software-pipelined across iterations: each batch's attention
            # overlaps the OTHER batch's QKV/va filler units on the in-order
            # PE queue; batch 0's QKV belongs to the NEXT iteration (filled
            # by the prologue for the first one).  x loads are issued a full
            # phase ahead of their consuming fillers so chunk arrival never
            # stalls the PE queue.
            if SCHED == "v2":
                gs, ge = [0, 0, 2, 6], [0, 2, 6, 12]
                emit_loads(1)
                for it in range(NI):
                    emit_att_it(0, it)
                    for g in range(gs[it], ge[it]):
                        emit_qkv_group(1, g)
                emit_va(1)
                emit_loads(0)
                for it in range(NI):
                    emit_att_it(1, it)
                    for g in range(gs[it], ge[it]):
                        emit_qkv_group(0, g)
                emit_va(0)
                flush_finish()
                return
            units1 = fill_units(1)
            emit_loads(0)
            for it in range(NI):
                emit_att_it(0, it)
                for u in range(UB[it], UB[it + 1]):
                    units1[u]()
            units0 = fill_units(0)
            emit_loads(1)
            for it in range(NI):
                emit_att_it(1, it)
                for u in range(UB[it], UB[it + 1]):
                    units0[u]()
            flush_finish()

        return prologue, body

    nc._dbg = {"qkv": qkv, "ot2s": ot2s, "va": va_t, "xts": xts}
    prologue, body = make_body()
    prologue()
    if niter >= 1:
        for _ in range(niter):
            body()
    else:
        with tc.For_i(0, -niter, 1) as iv:
            body(iv)

    for c in reversed(ctxs):
        c.__exit__(None, None, None)


def _build(mode, niter=1):
    key = (mode, niter)
    if key in _BUILD_CACHE:
        return _BUILD_CACHE[key]
    nc = bacc.Bacc("TRN2", target_bir_lowering=False, debug=False,
                   num_devices=N_CORES)
    t = {}
    t["xt"] = nc.dram_tensor("xt", (B, 128, KC * S), F16, kind="ExternalInput")
    t["wall"] = nc.dram_tensor("wall", (128, 3 * KC * DL), F16,
                               kind="ExternalInput")
    t["pwT"] = nc.dram_tensor("pwT", (DL, D), F16, kind="ExternalInput")
    t["rpr2T"] = nc.dram_tensor("rpr2T", (128, S), F16, kind="ExternalInput")
    t["misc"] = nc.dram_tensor("misc", (128, 4 * 128 + 1 + HD), F16,
                               kind="ExternalInput")
    if mode == "generic":
        t["maskT"] = nc.dram_tensor("maskT", (S, S), F32, kind="ExternalInput")
    t["y"] = nc.dram_tensor("y", (B, NI, 128, 4 * D), F16,
                            kind="ExternalOutput")

    with tile.TileContext(nc) as tc, \
            nc.allow_low_precision(reason="fp16 matmul operands"):
        _emit(nc, tc, t, mode, niter)
    nc.compile()
    _BUILD_CACHE[key] = (nc, t)
    return nc, t


def _prep_inputs(x, positions, causal_mask, wq, wk, wv, rpr, proj_w):
    """Host-side shard prep.  Returns (mode, per-core input maps)."""
    mask = np.asarray(causal_mask, np.float32).reshape(S, S)
    low = np.tril(np.ones((S, S), dtype=bool))
    if (mask[low] == 0.0).all() and (mask.any() and
                                     np.all(mask[~low] <= -1e6)):
        mode = "causal"
    elif not mask.any():
        mode = "zero"
    else:
        mode = "generic"

    # xt layout: (B, 128, KC*S): [b, p, k*S + s] = x[b, s, k*128 + p]
    xt = np.asarray(x, np.float32).transpose(0, 2, 1).reshape(B, KC, 128, S)
    xt = np.ascontiguousarray(xt.transpose(0, 2, 1, 3)).reshape(
        B, 128, KC * S).astype(np.float16)
    pos = np.asarray(positions).astype(np.int64)
    rpr_g = np.asarray(rpr, np.float32)[pos]  # (B, S, HD)
    rpr2 = np.ascontiguousarray(
        rpr_g.transpose(0, 2, 1)).reshape(B * HD, S).astype(np.float16)
    jj = np.arange(128)[:, None]
    ii = np.arange(128)[None, :]
    tril01 = (jj <= ii).astype(np.float16)
    ident = np.eye(128, dtype=np.float16)
    i2h = np.concatenate([np.eye(64), np.eye(64)], axis=1)
    i2 = np.concatenate([i2h, i2h], axis=0).astype(np.float16)
    trimask = ((jj > ii) * np.float32(-60000.0)).astype(np.float16)
    misc = np.concatenate(
        [tril01, ident, i2, np.ones((128, 1 + HD), np.float16), trimask],
        axis=1).astype(np.float16)
    maskT = np.ascontiguousarray(mask.T) if mode == "generic" else None

    wq = np.asarray(wq, np.float32)
    wk = np.asarray(wk, np.float32)
    wv = np.asarray(wv, np.float32)
    pw = np.asarray(proj_w, np.float32)

    def wall_of(wT):
        # [D, DL] -> [128, KC*DL]: col (k*DL + c), row r = wT[k*128 + r, c]
        return wT.reshape(KC, 128, DL).transpose(1, 0, 2).reshape(128,
                                                                  KC * DL)

    in_maps = []
    for c in range(N_CORES):
        rs = slice(c * DL, (c + 1) * DL)
        wall = np.concatenate(
            [wall_of(np.ascontiguousarray(w[rs, :].T))
             for w in (wq, wk, wv)], axis=1).astype(np.float16)
        m = {
            "xt": xt,
            "wall": np.ascontiguousarray(wall),
            "pwT": np.ascontiguousarray(pw[:, rs].T).astype(np.float16),
            "rpr2T": rpr2,
            "misc": misc,
        }
        if maskT is not None:
            m["maskT"] = maskT
        in_maps.append(m)
    return mode, in_maps


def kernel(x, positions, causal_mask, wq, wk, wv, rpr, proj_w, proj_b,
           _niter=1, **_ignored):
    mode, in_maps = _prep_inputs(x, positions, causal_mask, wq, wk, wv, rpr,
                                 proj_w)
    nc, _ = _build(mode, _niter)
    res = run_bass_kernel_spmd(nc, in_maps, core_ids=list(range(N_CORES)))
    out = np.zeros((B, S, D), dtype=np.float32)
    for r in res.results:
        # y layout: (B, NI, 128, 4*1024): [b, it, s, c*1024 + d]
        yr = r["y"].astype(np.float32).reshape(B, NI, 128, 4, D)
        out += yr.transpose(0, 1, 3, 2, 4).reshape(B, S, D)
    out += np.asarray(proj_b, np.float32)[None, None, :]
    return out



# revision 41
# speedup vs baseline: 1.0004x; 1.0004x over previous
"""Trainium2 Bass kernel for a 16-head causal MHA layer with relative-position
bias (B=2, S=2048, D=1024, H=16, HD=64), distributed over 8 NeuronCores.

Sharding: tensor parallel over heads — core c computes heads {2c, 2c+1} for
both batches.  The output projection is sharded over its input dim, so each
core returns a partial (B, S, D) output; the partials are summed on the host
(plus proj_b).

Per-core device pipeline (matmul operands in fp16):
  1. QKV projections in transposed layout: QT/KT/VT (128=2*HD, 2048) from
     per-k-chunk XT tiles x weight-slice chunks.  K' = K + rpr[positions]
     folded into the same PSUM accumulation group via a stacked-identity
     matmul, so the eviction is a plain copy.
  2. Attention per head in S^T layout: scores S^T(j,i) = K'^T-slice . Q-slice
     (both heads packed in one PE pass via tile_position row tiling, which
     runs concurrently on HW), exp on the scalar engine (scale folded in;
     the natural_log_exp_and_others act table is pre-loaded once so no
     per-use table reloads occur), causal masking via a 0/1-tril multiply
     on the exp OUTPUT, then OT_aug(65, i) += V_aug(j, 65)^T . P^T with a
     ones-column producing the softmax denominators for free.  Fully-masked
     j-blocks are skipped, fully-masked lead columns of diagonal blocks are
     not computed.
  3. Normalize (staged into the next block so no engine queue stalls):
     1/denom via a single custom-DVE reciprocal_approx_fast (fp32, via a
     small SBUF staging copy — the custom op mis-reads PSUM), partition-
     broadcast on gpsimd, then a DVE multiply reading OT_aug directly from
     PSUM.
  4. Output projection: y_partial(s, e) += OT2^T . pwT -> PSUM -> DVE/ACT
     eviction into a 4-block staging tile, one DMA per (b, it).

Scheduling: each batch's attention phase overlaps the OTHER batch's
QKV/V-transpose filler units on the in-order PE queue, interleaved in
next-phase consumption order; x loads are issued a phase ahead (per-chunk
DMAs/tiles); constants load in 4 consolidated DMAs (each dma_start costs
~650ns of SP-sequencer issue time).
"""

import sys

import numpy as np

try:
    import concourse.bass as bass  # noqa: F401
except ImportError:
    sys.path.insert(0, "/opt/trn_rl_repo")

import concourse.bass as bass
import concourse.mybir as mybir
import concourse.tile as tile
from concourse import bacc
from concourse.bass_utils import run_bass_kernel_spmd

B, S, D, H = 2, 2048, 1024, 16
HD = D // H  # 64
SCALE = HD**-0.5
N_CORES = 8
HPC = H // N_CORES  # heads per core = 2
DL = HPC * HD  # local head channels = 128
NJ = S // 128  # 16 j-chunks of 128
NI = S // 512  # 4 i-blocks of 512
KC = D // 128  # 8 contraction chunks of 128

F32 = mybir.dt.float32
F32R = mybir.dt.float32r
F16 = mybir.dt.float16

import os

# tunables
TRIL_ENGINE = os.environ.get("K_TRIL", "vector")  # causal 0/1 multiply engine
Y_TO_ACT = int(os.environ.get("K_YACT", "1"))  # y evictions to scalar, of 4
AV_SPLIT = bool(int(os.environ.get("K_AVSPLIT", "1")))  # defer masked AV
SC_TRIM = True  # skip fully-masked lead columns in the scores matmul
NORM_BC = os.environ.get("K_NORM", "gpsimd")  # "gpsimd" | "pe"
SC_SPLIT = bool(int(os.environ.get("K_SCSPLIT", "0")))  # per-head score tiles
PT_BUFS = int(os.environ.get("K_PTBUFS", "3"))  # exp-output ring depth
# (either way the multiply reads only one PSUM operand — a HW requirement)
SKIP_XT = bool(int(os.environ.get("K_SKIP_XT", "0")))  # ablation: no x loads
SKIP_Y = bool(int(os.environ.get("K_SKIP_Y", "0")))  # ablation: no y stores
HALF_EXP = bool(int(os.environ.get("K_HALF_EXP", "0")))  # ablation probe
HALF_AV = bool(int(os.environ.get("K_HALF_AV", "0")))  # ablation probe
NO_NORM = bool(int(os.environ.get("K_NO_NORM", "0")))  # ablation probe
NO_YEV = bool(int(os.environ.get("K_NO_YEV", "0")))  # ablation probe
NOREC = bool(int(os.environ.get("K_NOREC", "0")))  # ablation probe
NO_TRIL = bool(int(os.environ.get("K_NO_TRIL", "0")))  # ablation probe
PREMASK = bool(int(os.environ.get("K_PREMASK", "0")))  # -60000 add pre-exp
INLINE_NORM = bool(int(os.environ.get("K_INORM", "0")))  # rec/bc/mul inline
RPR_DVE = bool(int(os.environ.get("K_RPR_DVE", "0")))  # rpr add on DVE
VA_DMA = bool(int(os.environ.get("K_VA_DMA", "0")))  # V transpose via DMA
NORM_REC = os.environ.get("K_REC", "act")  # "act": 1/x = exp(-ln x); "dve"
NO_ATT = bool(int(os.environ.get("K_NO_ATT", "0")))  # ablation probe
NO_PROJ = bool(int(os.environ.get("K_NO_PROJ", "0")))  # ablation probe
SCHED = os.environ.get("K_SCHED", "v3")  # v2: end-of-phase va; v3: inline
UB = [int(x) for x in os.environ.get("K_UB", "0,2,5,10,16").split(",")]
QEV_ACT = bool(int(os.environ.get("K_QEV_ACT", "0")))  # qkv evict on ACT
TAIL_DEFER = bool(int(os.environ.get("K_TAIL", "0")))  # defer tail AVs

_BUILD_CACHE: dict = {}


def _emit(nc, tc, t, mode, niter):
    xt = t["xt"].ap()  # (B, 128, 8*2048) f16 (see _prep_inputs layout)
    wall = t["wall"].ap()  # (128, 3*KC*DL) f16: col (p*KC+k)*DL+c =
    #   w_p^T[k*128+r, c] — all three projection weights, one DMA
    pwT = t["pwT"].ap()  # (DL, D) f16
    rpr2 = t["rpr2T"].ap()  # (128, S) f16 : b-th 64 rows = rprT for batch b
    misc = t["misc"].ap()  # (128, 3*128+1) f16: [tril01 | ident | i2 | ones]
    y = t["y"].ap()  # (B, NI, 128, 4*1024) f16
    maskT = t["maskT"].ap() if "maskT" in t else None  # (S, S) f32

    # Pre-load the natural_log_exp_and_others act-function set (id 6): it
    # serves Exp, Ln AND Copy, so the compiler's table-load pass finds every
    # activation already satisfiable and inserts no per-use reloads (the
    # greedy per-func choice would otherwise toggle exp_and_others <->
    # natural_log around each Ln pair at 1.28us per reload).
    nc.scalar.add_instruction(mybir.InstLoadActFuncSet(
        act_func_set_id=6, name=nc.get_next_instruction_name(),
        engine=mybir.EngineType.Activation, ins=[], outs=[]))

    ctxs = [
        tc.tile_pool(name="consts", bufs=1),
        tc.tile_pool(name="xt", bufs=1),
        tc.tile_pool(name="qkv", bufs=1),
        tc.tile_pool(name="va", bufs=1),
        tc.tile_pool(name="pt", bufs=PT_BUFS),
        tc.tile_pool(name="sm", bufs=2),
        tc.tile_pool(name="ysb", bufs=2),
        tc.tile_pool(name="ps_mm", bufs=2, space="PSUM"),
        tc.tile_pool(name="ps_sc", bufs=4 if SC_SPLIT else 2, space="PSUM"),
        tc.tile_pool(name="ps_ot", bufs=2, space="PSUM"),
    ]
    if maskT is not None:
        ctxs.append(tc.tile_pool(name="mk", bufs=4))
    pools = [c.__enter__() for c in ctxs]
    (consts, xtp, qkvp, vap, ptp, smp, ysbp, ps_mm, ps_sc, ps_ot) = pools[:10]
    mkp = pools[10] if maskT is not None else None

    # --- persistent constants (loaded once, outside the timing loop).
    # Consolidated into 3 DMAs: each dma_start costs ~650ns of SP-sequencer
    # issue time, so 31 separate const loads would stall the prologue ~20us.
    wall_t = consts.tile([128, 3 * KC * DL], F16, tag="wall")
    nc.sync.dma_start(wall_t[:], wall)
    w_t = [[wall_t[:, (p * KC + k) * DL:(p * KC + k + 1) * DL]
            for k in range(KC)] for p in range(3)]
    pw_t = consts.tile([DL, D], F16, tag="pw")
    nc.sync.dma_start(pw_t[:], pwT)
    rpr_t = consts.tile([128, S], F16, tag="rpr")
    nc.sync.dma_start(rpr_t[:], rpr2)
    misc_t = consts.tile([128, 4 * 128 + 1 + HD], F16, tag="misc")
    nc.sync.dma_start(misc_t[:], misc)
    tril_t = misc_t[:, 0:128]
    id_t = misc_t[:, 128:256]
    i2_t = misc_t[:, 256:384]
    ones_t = misc_t[:, 384:385]
    ones1f_t = consts.tile([1, HD], F32, tag="ones1f")
    nc.vector.tensor_copy(ones1f_t[:], misc_t[0:1, 385:385 + HD])
    trimask_t = misc_t[:, 385 + HD:385 + HD + 128]  # strict-upper -60000

    # persistent V_aug tiles with the ones columns preset once:
    # layout [v_h0(0:64) | 1(64) | v_h1(65:129) | 1(129)]
    va_t = {(b, j): vap.tile([128, 2 * (HD + 1)], F16, tag=f"va{b}_{j}",
                             name=f"va{b}_{j}")
            for b in range(B) for j in range(NJ)}
    for (b, j), va in va_t.items():
        nc.vector.tensor_copy(va[:, HD:HD + 1], ones_t[:])
        nc.vector.tensor_copy(va[:, 2 * HD + 1:2 * HD + 2], ones_t[:])

    # persistent tiles referenced across loop iterations (bufs=1 tags).
    # x is held as one tile PER k-chunk so each chunk's DMA -> matmul
    # dependency is tracked separately (a single big tile would make the
    # first matmul wait for the whole 4MB load).
    xts = {(b, k): xtp.tile([128, S], F16, tag=f"xt{b}_{k}",
                            name=f"xt{b}_{k}")
           for b in range(B) for k in range(KC)}
    qkv = {b: (qkvp.tile([128, S], F16, tag=f"qt{b}", name=f"qt{b}"),
               qkvp.tile([128, S], F16, tag=f"kt{b}", name=f"kt{b}"),
               qkvp.tile([128, S], F16, tag=f"vt{b}", name=f"vt{b}"))
           for b in range(B)}
    ot2s = {b: qkvp.tile([128, S], F16, tag=f"ot2_{b}", name=f"ot2_{b}")
            for b in range(B)}

    def make_body():

        def emit_loads(b):
            if not SKIP_XT:
                # per-k-chunk DMAs so the first QKV matmul (which reads only
                # chunk 0) can start ~1/8 of the way into the transfer
                for k in range(KC):
                    nc.sync.dma_start(xts[(b, k)][:],
                                      xt[b, :, k * S:(k + 1) * S])

        def emit_qkv_group(b, g):
            # g in 0..11: projection p = g // NI, 512-col block sb = g % NI
            emit_qkv_psb(b, g // NI, g % NI)

        def emit_qkv_psb(b, p, sb):
            dst = qkv[b][p]
            ps = ps_mm.tile([128, 512], F32, tag="mm")
            for k in range(KC):
                nc.tensor.matmul(
                    ps[:], w_t[p][k][:],
                    xts[(b, k)][:, sb * 512:(sb + 1) * 512],
                    start=(k == 0),
                    stop=((p != 1 or RPR_DVE) and k == KC - 1))
            sl = slice(sb * 512, (sb + 1) * 512)
            if p == 1 and not RPR_DVE:
                # K' = K + rpr via a rank-64 stacked-identity matmul in the
                # same accumulation group (same bias for both head halves)
                nc.tensor.matmul(ps[:], i2_t[64 * b:64 * b + 64, :],
                                 rpr_t[64 * b:64 * b + 64, sl],
                                 start=False, stop=True)
            if p == 1 and RPR_DVE:
                # fold the rpr bias during eviction on the DVE instead of
                # spending PE cycles: one add per 64-row head half
                for hh in range(2):
                    nc.vector.tensor_add(
                        dst[64 * hh:64 * hh + 64, sl],
                        ps[64 * hh:64 * hh + 64, :],
                        rpr_t[64 * b:64 * b + 64, sl])
            elif QEV_ACT:
                nc.scalar.activation(dst[:, sl], ps[:],
                                     mybir.ActivationFunctionType.Copy)
            else:
                nc.vector.tensor_copy(dst[:, sl], ps[:])

        def emit_va(b):
            vt = qkv[b][2]
            for j in range(NJ):
                tp = ps_mm.tile([128, 128], F16, tag="mm", name="tp")
                nc.tensor.transpose(tp[:], vt[:, j * 128:(j + 1) * 128], id_t[:])
                va = va_t[(b, j)]
                nc.vector.tensor_copy(
                    va[:].rearrange("p (g x) -> p g x", g=2)[:, :, 0:HD],
                    tp[:].rearrange("p (g x) -> p g x", g=2))

        def emit_proj_blocks(b, sts, yt):
            if NO_PROJ:
                return None
            it = sts[0] // 4
            if yt is None:
                yt = ysbp.tile([128, 4 * D], F16, tag="y")
            last = (sts[-1] + 1) % 4 == 0
            for stq in sts:
                ssl = slice(stq * 128, (stq + 1) * 128)
                c = stq % 4
                for eb in range(D // 512):
                    pp = ps_mm.tile([128, 512], F32, tag="mm", name="pp")
                    nc.tensor.matmul(
                        pp[:], ot2s[b][:, ssl],
                        pw_t[:, eb * 512:(eb + 1) * 512],
                        start=True, stop=True)
                    ysl = slice(c * D + eb * 512, c * D + (eb + 1) * 512)
                    if NO_YEV:
                        continue
                    if (2 * c + eb) % 4 < Y_TO_ACT:
                        nc.scalar.activation(
                            yt[:, ysl], pp[:],
                            mybir.ActivationFunctionType.Copy)
                    else:
                        nc.vector.tensor_copy(yt[:, ysl], pp[:])
            if last and not (SKIP_Y or NO_YEV):
                nc.sync.dma_start(y[b, it], yt[:])
            return yt

        def emit_att_stub(b, it):
            isl = slice(it * 512, (it + 1) * 512)
            nc.vector.tensor_copy(ot2s[b][:, isl], rpr_t[:, 0:512])
            emit_proj_blocks(b, range(4 * it, 4 * it + 4), None)

        pending_stages = []
        av_tail = []  # deferred trailing AV matmuls (TAIL_DEFER)

        def pump(n=1):
            for _ in range(n):
                if pending_stages:
                    pending_stages.pop(0)()

        def flush_finish():
            pump(len(pending_stages))

        def emit_att_it(b, it):
            if NO_ATT:
                emit_att_stub(b, it)
                return
            # scores -> exp -> AV for one 512-wide query block, software-
            # pipelined two j-chunks deep so the in-order PE queue is not
            # gated on the ACT exp latency each chunk.  The normalize +
            # projection of each block is DEFERRED into the next block's
            # scores phase so the PE never stalls on the norm chain.
            qt, kt, _ = qkv[b]
            isl = slice(it * 512, (it + 1) * 512)
            jhi = (4 * it + 3) if mode == "causal" else (NJ - 1)
            otp = [ps_ot.tile([HD + 1, 512], F32, tag="ot", name=f"ot{_h}")
                   for _h in range(HPC)]

            def emit_scores(j):
                jsl = slice(j * 128, (j + 1) * 128)
                dc = max(0, (j - 4 * it) * 128) if mode == "causal" else 0
                dct = dc if SC_TRIM else 0
                if SC_SPLIT:
                    tiles = [ps_sc.tile([128, 512], F32, tag="sc",
                                        name=f"sc{_h}") for _h in range(HPC)]
                    views = [lambda lo, hi, t=t: t[:, lo:hi] for t in tiles]
                    whole = None
                else:
                    sc2 = ps_sc.tile([128, 2 * 512], F32, tag="sc",
                                     name="sc2")
                    views = [
                        lambda lo, hi, h=h: sc2[:, h * 512 + lo:h * 512 + hi]
                        for h in range(HPC)]
                    whole = sc2
                for h in range(HPC):
                    hsl = slice(h * HD, (h + 1) * HD)
                    nc.tensor.matmul(
                        views[h](dct, 512), kt[hsl, jsl],
                        qt[hsl, isl.start + dct:isl.stop], start=True,
                        stop=True, tile_position=(h * HD, 0))
                if maskT is not None:
                    mkt = mkp.tile([128, 512], F32, tag="mk")
                    nc.sync.dma_start(mkt[:], maskT[jsl, isl])
                    for h in range(HPC):
                        nc.vector.tensor_add(
                            views[h](0, 512), views[h](0, 512), mkt[:])
                if PREMASK and mode == "causal" and j >= 4 * it:
                    # additive causal mask on the diagonal 128-block of the
                    # scores PSUM, BEFORE the exp (same semantics as the
                    # reference).  Runs ~2 j-chunks ahead of the exp, so the
                    # DVE hop is off the exp->AV critical path (unlike the
                    # 0/1-tril multiply on the exp output it replaces).
                    for h in range(HPC):
                        nc.vector.tensor_add(
                            views[h](dc, dc + 128), views[h](dc, dc + 128),
                            trimask_t)
                return views, whole

            def emit_exp_av(j, sc):
                views, whole = sc
                dc = max(0, (j - 4 * it) * 128) if mode == "causal" else 0
                diag = mode == "causal" and j >= 4 * it
                pt2 = ptp.tile([128, 2 * 512], F16, tag="pt", name="pt2")
                if HALF_EXP:
                    # timing probe: exp only head 0; head 1 reads h0's P
                    nc.scalar.activation(
                        pt2[:, dc:512], views[0](dc, 512),
                        mybir.ActivationFunctionType.Exp, scale=SCALE)
                elif dc == 0 and whole is not None:
                    nc.scalar.activation(
                        pt2[:], whole[:],
                        mybir.ActivationFunctionType.Exp, scale=SCALE)
                else:
                    for h in range(HPC):
                        nc.scalar.activation(
                            pt2[:, h * 512 + dc:(h + 1) * 512],
                            views[h](dc, 512),
                            mybir.ActivationFunctionType.Exp, scale=SCALE)
                va = va_t[(b, j)]

                def ptb(h):
                    return 0 if HALF_EXP else h * 512

                class _NopEng:
                    def tensor_mul(self, *a, **k):
                        pass
                eng = (_NopEng() if (NO_TRIL or PREMASK) else
                       nc.gpsimd if TRIL_ENGINE == "pool" else nc.vector)
                if diag and AV_SPLIT and not PREMASK and j > 0:
                    # the diagonal 128x128 block needs the 0/1-tril multiply
                    # on the exp output; AV over the unmasked columns runs
                    # immediately, the masked-block AV trails off-path (the
                    # accumulation order into otp does not matter)
                    for h in range(HPC):
                        if dc + 128 < 512:
                            nc.tensor.matmul(
                                otp[h][:, dc + 128:512],
                                va[:, h * (HD + 1):(h + 1) * (HD + 1)],
                                pt2[:, ptb(h) + dc + 128:ptb(h) + 512],
                                start=(j == 0), stop=(j == jhi),
                                skip_group_check=True)
                        if not (HALF_EXP and h):
                            eng.tensor_mul(
                                pt2[:, ptb(h) + dc:ptb(h) + dc + 128],
                                pt2[:, ptb(h) + dc:ptb(h) + dc + 128],
                                tril_t[:])
                        nc.tensor.matmul(
                            otp[h][:, dc:dc + 128],
                            va[:, h * (HD + 1):(h + 1) * (HD + 1)],
                            pt2[:, ptb(h) + dc:ptb(h) + dc + 128],
                            start=(j == 0), stop=(j == jhi),
                            skip_group_check=True)
                else:
                    if diag:
                        for h in range(HPC):
                            if HALF_EXP and h:
                                continue
                            eng.tensor_mul(
                                pt2[:, ptb(h) + dc:ptb(h) + dc + 128],
                                pt2[:, ptb(h) + dc:ptb(h) + dc + 128],
                                tril_t[:])
                    for h in range(1 if HALF_AV else HPC):
                        nc.tensor.matmul(
                            otp[h][:, dc:512],
                            va[:, h * (HD + 1):(h + 1) * (HD + 1)],
                            pt2[:, ptb(h) + dc:ptb(h) + 512],
                            start=(j == 0), stop=(j == jhi),
                            skip_group_check=True)

            def emit_exp_tail(j, sc):
                # exp + tril only (no AV) for a trailing chunk; the AV is
                # deferred until the NEXT block's first scores are queued,
                # so a stalled AV never gates them on the in-order PE queue
                views, whole = sc
                dc = max(0, (j - 4 * it) * 128) if mode == "causal" else 0
                diag = mode == "causal" and j >= 4 * it
                pt2 = ptp.tile([128, 2 * 512], F16, tag="pt", name="pt2")
                if dc == 0 and whole is not None:
                    nc.scalar.activation(
                        pt2[:], whole[:],
                        mybir.ActivationFunctionType.Exp, scale=SCALE)
                else:
                    for h in range(HPC):
                        nc.scalar.activation(
                            pt2[:, h * 512 + dc:(h + 1) * 512],
                            views[h](dc, 512),
                            mybir.ActivationFunctionType.Exp, scale=SCALE)
                if diag and not (NO_TRIL or PREMASK):
                    eng = (nc.gpsimd if TRIL_ENGINE == "pool" else nc.vector)
                    for h in range(HPC):
                        eng.tensor_mul(
                            pt2[:, h * 512 + dc:h * 512 + dc + 128],
                            pt2[:, h * 512 + dc:h * 512 + dc + 128],
                            tril_t[:])
                return pt2, dc

            def emit_av_tail(j, pt2, dc):
                va = va_t[(b, j)]
                for h in range(HPC):
                    nc.tensor.matmul(
                        otp[h][:, dc:512],
                        va[:, h * (HD + 1):(h + 1) * (HD + 1)],
                        pt2[:, h * 512 + dc:(h + 1) * 512],
                        start=(j == 0), stop=(j == jhi),
                        skip_group_check=True)

            pend = [(0, emit_scores(0))]
            if jhi >= 1:
                pend.append((1, emit_scores(1)))
            # previous block's deferred tail AVs go behind our first scores
            for fn in av_tail:
                fn()
            del av_tail[:]
            pump()
            for j in range(2, jhi + 1):
                pend.append((j, emit_scores(j)))
                pump()
                jd, scd = pend.pop(0)
                emit_exp_av(jd, scd)
            flush_finish()
            if TAIL_DEFER and mode == "causal" and not (HALF_EXP or HALF_AV):
                for jd, scd in pend:
                    pt2d, dcd = emit_exp_tail(jd, scd)
                    av_tail.append(
                        lambda jd=jd, p=pt2d, d=dcd: emit_av_tail(jd, p, d))
            else:
                for jd, scd in pend:
                    emit_exp_av(jd, scd)

            # the normalize + projection chain is emitted as STAGES pumped
            # one scores-chunk apart during the NEXT block, so each hop's
            # input is complete before its (in-order) engine reaches it:
            # recs -> broadcasts -> multiplies -> proj(st01) -> proj(st23).
            # The OT_aug PSUM tiles are read directly by the rec + mul
            # stages (no SBUF eviction); the ps_ot ring (bufs=2) keeps the
            # next block in the other slot, and stage_mul retires this slot
            # well before block it+2 needs it.
            st = {}

            def stage_rec():
                st["rec"] = []
                for h in range(HPC):
                    op = otp[0] if HALF_AV else otp[h]
                    dn = smp.tile([1, 512], F32, tag="dn")
                    rec = smp.tile([1, 512], F32, tag="rec")
                    if not NO_NORM:
                        # ~18-bit 1/x in a single custom-DVE op; keeps the
                        # scalar engine free for the softmax exps.  The
                        # denom row goes through SBUF first — the custom op
                        # mis-reads PSUM sources.
                        nc.vector.tensor_copy(dn[:], op[HD:HD + 1, :])
                        if NOREC:
                            nc.vector.tensor_copy(rec[:], dn[:])
                        else:
                            nc.vector.reciprocal_approx_fast(rec[:], dn[:])
                    st["rec"].append(rec)

            def stage_bc():
                st["bc"] = []
                for h in range(HPC):
                    if NORM_BC == "bcast":
                        st["bc"].append(None)
                        continue
                    bc = smp.tile([HD, 512],
                                  F16 if NORM_BC == "pe" else F32, tag="bc")
                    if not NO_NORM:
                        if NORM_BC == "pe":
                            # rank-1 outer product ones^T x rec on the PE
                            # (f32r views: 1 cyc/row); gpsimd
                            # partition_broadcast can be slow on HW
                            bcp = ps_mm.tile([HD, 512], F32, tag="mm",
                                             name="bcp")
                            nc.tensor.matmul(bcp[:],
                                             ones1f_t[:].bitcast(F32R),
                                             st["rec"][h][:].bitcast(F32R),
                                             start=True, stop=True)
                            nc.vector.tensor_copy(bc[:], bcp[:])
                        else:
                            nc.gpsimd.partition_broadcast(
                                bc[:], st["rec"][h][:])
                    st["bc"].append(bc)

            def stage_mul():
                for h in range(HPC):
                    op = otp[0] if HALF_AV else otp[h]
                    if NO_NORM:
                        src = rpr_t[0:HD, 0:512]
                    elif NORM_BC == "bcast":
                        src = st["rec"][h][:].broadcast_to([HD, 512])
                    else:
                        src = st["bc"][h][:]
                    nc.vector.tensor_mul(
                        ot2s[b][h * HD:(h + 1) * HD, isl], op[0:HD, :],
                        src)

            def stage_proj01():
                st["yt"] = emit_proj_blocks(b, range(4 * it, 4 * it + 2),
                                            None)

            def stage_proj23():
                emit_proj_blocks(b, range(4 * it + 2, 4 * it + 4), st["yt"])

            if INLINE_NORM:
                # rec/bc/mul touch only DVE+Pool; emitting them inline at
                # block end starts the chain earlier and frees the ot PSUM
                # ring sooner.  Only the PE-bound proj stages stay deferred.
                stage_rec()
                stage_bc()
                stage_mul()
                pending_stages.extend(
                    [lambda: None, stage_proj01, stage_proj23])
            else:
                pending_stages.extend(
                    [stage_rec, stage_bc, stage_mul, stage_proj01,
                     stage_proj23])

        def emit_va_set(b, sb):
            # V^T -> va transposes for j-chunks 4sb..4sb+3 (follows the
            # (2, sb) projection group that produced those vt columns)
            vt = qkv[b][2]
            for j in range(4 * sb, 4 * sb + 4):
                va = va_t[(b, j)]
                if VA_DMA:
                    # DMA xbar transpose to a 2D scratch (the xbar can't
                    # target the interleaved va view), then DVE-interleave;
                    # saves the PE pass + PSUM ring slot
                    tps = smp.tile([128, 128], F16, tag="vtp", name="vtp")
                    nc.sync.dma_start_transpose(
                        tps[:], vt[:, j * 128:(j + 1) * 128])
                    nc.vector.tensor_copy(
                        va[:].rearrange("p (g x) -> p g x", g=2)[:, :, 0:HD],
                        tps[:].rearrange("p (g x) -> p g x", g=2))
                    continue
                tp = ps_mm.tile([128, 128], F16, tag="mm", name="tp")
                nc.tensor.transpose(tp[:], vt[:, j * 128:(j + 1) * 128],
                                    id_t[:])
                nc.vector.tensor_copy(
                    va[:].rearrange("p (g x) -> p g x", g=2)[:, :, 0:HD],
                    tp[:].rearrange("p (g x) -> p g x", g=2))

        def fill_units(b):
            # filler work in next-phase consumption order: (q,k,v,va) per
            # 512-col block, so att(b, it) of the next phase finds qt/kt/va
            # for its own window already complete, and the va copies are
            # spread across the phase instead of bursting at its end
            units = []
            for sb in range(NI):
                for p in range(3):
                    units.append(
                        lambda b=b, p=p, sb=sb: emit_qkv_psb(b, p, sb))
                units.append(lambda b=b, sb=sb: emit_va_set(b, sb))
            return units

        def prologue():
            # one-time pipeline fill: batch 0's QKV ahead of the loop, and
            # batch 1's x staged so the first body's fillers don't wait.
            # loads(1) is issued AFTER the QKV emission: the 8 DMA queues
            # share bandwidth, so batching both 4MB loads up front would
            # delay batch 0's first chunk (and the whole pipeline) 2x.
            emit_loads(0)
            for g in range(3 * NI):
                emit_qkv_group(0, g)
            emit_loads(1)
            emit_va(0)

        def body(_iv=None):
            # Software-pipelined across iterations: each batch's attention
            # overlaps the OTHER batch's QKV/va filler units on the in-order
            # PE queue; batch 0's QKV belongs to the NEXT iteration (filled
            # by the prologue for the first one).  x loads are issued a full
            # phase ahead of their consuming fillers so chunk arrival never
            # stalls the PE queue.
            if SCHED == "v2":
                gs, ge = [0, 0, 2, 6], [0, 2, 6, 12]
                emit_loads(1)
                for it in range(NI):
                    emit_att_it(0, it)
                    for g in range(gs[it], ge[it]):
                        emit_qkv_group(1, g)
                emit_va(1)
                emit_loads(0)
                for it in range(NI):
                    emit_att_it(1, it)
                    for g in range(gs[it], ge[it]):
                        emit_qkv_group(0, g)
                emit_va(0)
                for fn in av_tail:
                    fn()
                del av_tail[:]
                flush_finish()
                return
            units1 = fill_units(1)
            emit_loads(0)
            for it in range(NI):
                emit_att_it(0, it)
                for u in range(UB[it], UB[it + 1]):
                    units1[u]()
            units0 = fill_units(0)
            emit_loads(1)
            for it in range(NI):
                emit_att_it(1, it)
                for u in range(UB[it], UB[it + 1]):
                    units0[u]()
            for fn in av_tail:
                fn()
            del av_tail[:]
            flush_finish()

        return prologue, body

    nc._dbg = {"qkv": qkv, "ot2s": ot2s, "va": va_t, "xts": xts}
    prologue, body = make_body()
    prologue()
    if niter >= 1:
        for _ in range(niter):
            body()
    else:
        with tc.For_i(0, -niter, 1) as iv:
            body(iv)

    for c in reversed(ctxs):
        c.__exit__(None, None, None)


def _build(mode, niter=1):
    key = (mode, niter)
    if key in _BUILD_CACHE:
        return _BUILD_CACHE[key]
    nc = bacc.Bacc("TRN2", target_bir_lowering=False, debug=False,
                   num_devices=N_CORES)
    t = {}
    t["xt"] = nc.dram_tensor("xt", (B, 128, KC * S), F16, kind="ExternalInput")
    t["wall"] = nc.dram_tensor("wall", (128, 3 * KC * DL), F16,
                               kind="ExternalInput")
    t["pwT"] = nc.dram_tensor("pwT", (DL, D), F16, kind="ExternalInput")
    t["rpr2T"] = nc.dram_tensor("rpr2T", (128, S), F16, kind="ExternalInput")
    t["misc"] = nc.dram_tensor("misc", (128, 4 * 128 + 1 + HD), F16,
                               kind="ExternalInput")
    if mode == "generic":
        t["maskT"] = nc.dram_tensor("maskT", (S, S), F32, kind="ExternalInput")
    t["y"] = nc.dram_tensor("y", (B, NI, 128, 4 * D), F16,
                            kind="ExternalOutput")

    with tile.TileContext(nc) as tc, \
            nc.allow_low_precision(reason="fp16 matmul operands"):
        _emit(nc, tc, t, mode, niter)
    nc.compile()
    _BUILD_CACHE[key] = (nc, t)
    return nc, t


def _prep_inputs(x, positions, causal_mask, wq, wk, wv, rpr, proj_w):
    """Host-side shard prep.  Returns (mode, per-core input maps)."""
    mask = np.asarray(causal_mask, np.float32).reshape(S, S)
    low = np.tril(np.ones((S, S), dtype=bool))
    if (mask[low] == 0.0).all() and (mask.any() and
                                     np.all(mask[~low] <= -1e6)):
        mode = "causal"
    elif not mask.any():
        mode = "zero"
    else:
        mode = "generic"

    # xt layout: (B, 128, KC*S): [b, p, k*S + s] = x[b, s, k*128 + p]
    xt = np.asarray(x, np.float32).transpose(0, 2, 1).reshape(B, KC, 128, S)
    xt = np.ascontiguousarray(xt.transpose(0, 2, 1, 3)).reshape(
        B, 128, KC * S).astype(np.float16)
    pos = np.asarray(positions).astype(np.int64)
    rpr_g = np.asarray(rpr, np.float32)[pos]  # (B, S, HD)
    rpr2 = np.ascontiguousarray(
        rpr_g.transpose(0, 2, 1)).reshape(B * HD, S).astype(np.float16)
    jj = np.arange(128)[:, None]
    ii = np.arange(128)[None, :]
    tril01 = (jj <= ii).astype(np.float16)
    ident = np.eye(128, dtype=np.float16)
    i2h = np.concatenate([np.eye(64), np.eye(64)], axis=1)
    i2 = np.concatenate([i2h, i2h], axis=0).astype(np.float16)
    trimask = ((jj > ii) * np.float32(-60000.0)).astype(np.float16)
    misc = np.concatenate(
        [tril01, ident, i2, np.ones((128, 1 + HD), np.float16), trimask],
        axis=1).astype(np.float16)
    maskT = np.ascontiguousarray(mask.T) if mode == "generic" else None

    wq = np.asarray(wq, np.float32)
    wk = np.asarray(wk, np.float32)
    wv = np.asarray(wv, np.float32)
    pw = np.asarray(proj_w, np.float32)

    def wall_of(wT):
        # [D, DL] -> [128, KC*DL]: col (k*DL + c), row r = wT[k*128 + r, c]
        return wT.reshape(KC, 128, DL).transpose(1, 0, 2).reshape(128,
                                                                  KC * DL)

    in_maps = []
    for c in range(N_CORES):
        rs = slice(c * DL, (c + 1) * DL)
        wall = np.concatenate(
            [wall_of(np.ascontiguousarray(w[rs, :].T))
             for w in (wq, wk, wv)], axis=1).astype(np.float16)
        m = {
            "xt": xt,
            "wall": np.ascontiguousarray(wall),
            "pwT": np.ascontiguousarray(pw[:, rs].T).astype(np.float16),
            "rpr2T": rpr2,
            "misc": misc,
        }
        if maskT is not None:
            m["maskT"] = maskT
        in_maps.append(m)
    return mode, in_maps


def kernel(x, positions, causal_mask, wq, wk, wv, rpr, proj_w, proj_b,
           _niter=1, **_ignored):
    mode, in_maps = _prep_inputs(x, positions, causal_mask, wq, wk, wv, rpr,
                                 proj_w)
    nc, _ = _build(mode, _niter)
    res = run_bass_kernel_spmd(nc, in_maps, core_ids=list(range(N_CORES)))
    out = np.zeros((B, S, D), dtype=np.float32)
    for r in res.results:
        # y layout: (B, NI, 128, 4*1024): [b, it, s, c*1024 + d]
        yr = r["y"].astype(np.float32).reshape(B, NI, 128, 4, D)
        out += yr.transpose(0, 1, 3, 2, 4).reshape(B, S, D)
    out += np.asarray(proj_b, np.float32)[None, None, :]
    return out



# revision 43
# speedup vs baseline: 1.0054x; 1.0050x over previous
"""Trainium2 Bass kernel for a 16-head causal MHA layer with relative-position
bias (B=2, S=2048, D=1024, H=16, HD=64), distributed over 8 NeuronCores.

Sharding: tensor parallel over heads — core c computes heads {2c, 2c+1} for
both batches.  The output projection is sharded over its input dim, so each
core returns a partial (B, S, D) output; the partials are summed on the host
(plus proj_b).

Per-core device pipeline (matmul operands in fp16):
  1. QKV projections in transposed layout: QT/KT/VT (128=2*HD, 2048) from
     per-k-chunk XT tiles x weight-slice chunks.  K' = K + rpr[positions]
     folded into the same PSUM accumulation group via a stacked-identity
     matmul, so the eviction is a plain copy.
  2. Attention per head in S^T layout: scores S^T(j,i) = K'^T-slice . Q-slice
     (both heads packed in one PE pass via tile_position row tiling, which
     runs concurrently on HW), exp on the scalar engine (scale folded in;
     the natural_log_exp_and_others act table is pre-loaded once so no
     per-use table reloads occur), causal masking via a 0/1-tril multiply
     on the exp OUTPUT, then OT_aug(65, i) += V_aug(j, 65)^T . P^T with a
     ones-column producing the softmax denominators for free.  Fully-masked
     j-blocks are skipped, fully-masked lead columns of diagonal blocks are
     not computed.
  3. Normalize (staged into the next block so no engine queue stalls):
     1/denom via a single custom-DVE reciprocal_approx_fast (fp32, via a
     small SBUF staging copy — the custom op mis-reads PSUM), partition-
     broadcast on gpsimd, then a DVE multiply reading OT_aug directly from
     PSUM.
  4. Output projection: y_partial(s, e) += OT2^T . pwT -> PSUM -> DVE/ACT
     eviction into a 4-block staging tile, one DMA per (b, it).

Scheduling: each batch's attention phase overlaps the OTHER batch's
QKV/V-transpose filler units on the in-order PE queue, interleaved in
next-phase consumption order; x loads are issued a phase ahead (per-chunk
DMAs/tiles); constants load in 4 consolidated DMAs (each dma_start costs
~650ns of SP-sequencer issue time).
"""

import sys

import numpy as np

try:
    import concourse.bass as bass  # noqa: F401
except ImportError:
    sys.path.insert(0, "/opt/trn_rl_repo")

import concourse.bass as bass
import concourse.mybir as mybir
import concourse.tile as tile
from concourse import bacc
from concourse.bass_utils import run_bass_kernel_spmd

B, S, D, H = 2, 2048, 1024, 16
HD = D // H  # 64
SCALE = HD**-0.5
N_CORES = 8
HPC = H // N_CORES  # heads per core = 2
DL = HPC * HD  # local head channels = 128
NJ = S // 128  # 16 j-chunks of 128
NI = S // 512  # 4 i-blocks of 512
KC = D // 128  # 8 contraction chunks of 128

F32 = mybir.dt.float32
F32R = mybir.dt.float32r
F16 = mybir.dt.float16

import os

# tunables
TRIL_ENGINE = os.environ.get("K_TRIL", "vector")  # causal 0/1 multiply engine
Y_TO_ACT = int(os.environ.get("K_YACT", "1"))  # y evictions to scalar, of 4
AV_SPLIT = bool(int(os.environ.get("K_AVSPLIT", "1")))  # defer masked AV
SC_TRIM = True  # skip fully-masked lead columns in the scores matmul
NORM_BC = os.environ.get("K_NORM", "gpsimd")  # "gpsimd" | "pe"
SC_SPLIT = bool(int(os.environ.get("K_SCSPLIT", "0")))  # per-head score tiles
PT_BUFS = int(os.environ.get("K_PTBUFS", "3"))  # exp-output ring depth
# (either way the multiply reads only one PSUM operand — a HW requirement)
SKIP_XT = bool(int(os.environ.get("K_SKIP_XT", "0")))  # ablation: no x loads
SKIP_Y = bool(int(os.environ.get("K_SKIP_Y", "0")))  # ablation: no y stores
HALF_EXP = bool(int(os.environ.get("K_HALF_EXP", "0")))  # ablation probe
HALF_AV = bool(int(os.environ.get("K_HALF_AV", "0")))  # ablation probe
NO_NORM = bool(int(os.environ.get("K_NO_NORM", "0")))  # ablation probe
NO_YEV = bool(int(os.environ.get("K_NO_YEV", "0")))  # ablation probe
NOREC = bool(int(os.environ.get("K_NOREC", "0")))  # ablation probe
NO_TRIL = bool(int(os.environ.get("K_NO_TRIL", "0")))  # ablation probe
PREMASK = bool(int(os.environ.get("K_PREMASK", "0")))  # -60000 add pre-exp
INLINE_NORM = bool(int(os.environ.get("K_INORM", "0")))  # rec/bc/mul inline
RPR_DVE = bool(int(os.environ.get("K_RPR_DVE", "0")))  # rpr add on DVE
VA_DMA = bool(int(os.environ.get("K_VA_DMA", "0")))  # V transpose via DMA
NORM_REC = os.environ.get("K_REC", "act")  # "act": 1/x = exp(-ln x); "dve"
NO_ATT = bool(int(os.environ.get("K_NO_ATT", "0")))  # ablation probe
NO_PROJ = bool(int(os.environ.get("K_NO_PROJ", "0")))  # ablation probe
SCHED = os.environ.get("K_SCHED", "v3")  # v2: end-of-phase va; v3: inline
UB = [int(x) for x in os.environ.get("K_UB", "0,2,5,10,16").split(",")]
QEV_ACT = bool(int(os.environ.get("K_QEV_ACT", "0")))  # qkv evict on ACT
TAIL_DEFER = bool(int(os.environ.get("K_TAIL", "0")))  # defer tail AVs
ESPLIT = bool(int(os.environ.get("K_ESPLIT", "0")))  # per-head exp ops

_BUILD_CACHE: dict = {}


def _emit(nc, tc, t, mode, niter):
    xt = t["xt"].ap()  # (B, 128, 8*2048) f16 (see _prep_inputs layout)
    wall = t["wall"].ap()  # (128, 3*KC*DL) f16: col (p*KC+k)*DL+c =
    #   w_p^T[k*128+r, c] — all three projection weights, one DMA
    pwT = t["pwT"].ap()  # (DL, D) f16
    rpr2 = t["rpr2T"].ap()  # (128, S) f16 : b-th 64 rows = rprT for batch b
    misc = t["misc"].ap()  # (128, 3*128+1) f16: [tril01 | ident | i2 | ones]
    y = t["y"].ap()  # (B, NI, 128, 4*1024) f16
    maskT = t["maskT"].ap() if "maskT" in t else None  # (S, S) f32

    # Pre-load the natural_log_exp_and_others act-function set (id 6): it
    # serves Exp, Ln AND Copy, so the compiler's table-load pass finds every
    # activation already satisfiable and inserts no per-use reloads (the
    # greedy per-func choice would otherwise toggle exp_and_others <->
    # natural_log around each Ln pair at 1.28us per reload).
    nc.scalar.add_instruction(mybir.InstLoadActFuncSet(
        act_func_set_id=6, name=nc.get_next_instruction_name(),
        engine=mybir.EngineType.Activation, ins=[], outs=[]))

    ctxs = [
        tc.tile_pool(name="consts", bufs=1),
        tc.tile_pool(name="xt", bufs=1),
        tc.tile_pool(name="qkv", bufs=1),
        tc.tile_pool(name="va", bufs=1),
        tc.tile_pool(name="pt", bufs=PT_BUFS),
        tc.tile_pool(name="sm", bufs=2),
        tc.tile_pool(name="ysb", bufs=2),
        tc.tile_pool(name="ps_mm", bufs=2, space="PSUM"),
        tc.tile_pool(name="ps_sc", bufs=4 if SC_SPLIT else 2, space="PSUM"),
        tc.tile_pool(name="ps_ot", bufs=2, space="PSUM"),
    ]
    if maskT is not None:
        ctxs.append(tc.tile_pool(name="mk", bufs=4))
    pools = [c.__enter__() for c in ctxs]
    (consts, xtp, qkvp, vap, ptp, smp, ysbp, ps_mm, ps_sc, ps_ot) = pools[:10]
    mkp = pools[10] if maskT is not None else None

    # --- persistent constants (loaded once, outside the timing loop).
    # Consolidated into 3 DMAs: each dma_start costs ~650ns of SP-sequencer
    # issue time, so 31 separate const loads would stall the prologue ~20us.
    wall_t = consts.tile([128, 3 * KC * DL], F16, tag="wall")
    nc.sync.dma_start(wall_t[:], wall)
    w_t = [[wall_t[:, (p * KC + k) * DL:(p * KC + k + 1) * DL]
            for k in range(KC)] for p in range(3)]
    pw_t = consts.tile([DL, D], F16, tag="pw")
    nc.sync.dma_start(pw_t[:], pwT)
    rpr_t = consts.tile([128, S], F16, tag="rpr")
    nc.sync.dma_start(rpr_t[:], rpr2)
    misc_t = consts.tile([128, 4 * 128 + 1 + HD], F16, tag="misc")
    nc.sync.dma_start(misc_t[:], misc)
    tril_t = misc_t[:, 0:128]
    id_t = misc_t[:, 128:256]
    i2_t = misc_t[:, 256:384]
    ones_t = misc_t[:, 384:385]
    ones1f_t = consts.tile([1, HD], F32, tag="ones1f")
    nc.vector.tensor_copy(ones1f_t[:], misc_t[0:1, 385:385 + HD])
    trimask_t = misc_t[:, 385 + HD:385 + HD + 128]  # strict-upper -60000

    # persistent V_aug tiles with the ones columns preset once:
    # layout [v_h0(0:64) | 1(64) | v_h1(65:129) | 1(129)]
    va_t = {(b, j): vap.tile([128, 2 * (HD + 1)], F16, tag=f"va{b}_{j}",
                             name=f"va{b}_{j}")
            for b in range(B) for j in range(NJ)}
    for (b, j), va in va_t.items():
        nc.vector.tensor_copy(va[:, HD:HD + 1], ones_t[:])
        nc.vector.tensor_copy(va[:, 2 * HD + 1:2 * HD + 2], ones_t[:])

    # persistent tiles referenced across loop iterations (bufs=1 tags).
    # x is held as one tile PER k-chunk so each chunk's DMA -> matmul
    # dependency is tracked separately (a single big tile would make the
    # first matmul wait for the whole 4MB load).
    xts = {(b, k): xtp.tile([128, S], F16, tag=f"xt{b}_{k}",
                            name=f"xt{b}_{k}")
           for b in range(B) for k in range(KC)}
    qkv = {b: (qkvp.tile([128, S], F16, tag=f"qt{b}", name=f"qt{b}"),
               qkvp.tile([128, S], F16, tag=f"kt{b}", name=f"kt{b}"),
               qkvp.tile([128, S], F16, tag=f"vt{b}", name=f"vt{b}"))
           for b in range(B)}
    ot2s = {b: qkvp.tile([128, S], F16, tag=f"ot2_{b}", name=f"ot2_{b}")
            for b in range(B)}

    def make_body():

        def emit_loads(b):
            if not SKIP_XT:
                # per-k-chunk DMAs so the first QKV matmul (which reads only
                # chunk 0) can start ~1/8 of the way into the transfer
                for k in range(KC):
                    nc.sync.dma_start(xts[(b, k)][:],
                                      xt[b, :, k * S:(k + 1) * S])

        def emit_qkv_group(b, g):
            # g in 0..11: projection p = g // NI, 512-col block sb = g % NI
            emit_qkv_psb(b, g // NI, g % NI)

        def emit_qkv_psb(b, p, sb):
            dst = qkv[b][p]
            ps = ps_mm.tile([128, 512], F32, tag="mm")
            for k in range(KC):
                nc.tensor.matmul(
                    ps[:], w_t[p][k][:],
                    xts[(b, k)][:, sb * 512:(sb + 1) * 512],
                    start=(k == 0),
                    stop=((p != 1 or RPR_DVE) and k == KC - 1))
            sl = slice(sb * 512, (sb + 1) * 512)
            if p == 1 and not RPR_DVE:
                # K' = K + rpr via a rank-64 stacked-identity matmul in the
                # same accumulation group (same bias for both head halves)
                nc.tensor.matmul(ps[:], i2_t[64 * b:64 * b + 64, :],
                                 rpr_t[64 * b:64 * b + 64, sl],
                                 start=False, stop=True)
            if p == 1 and RPR_DVE:
                # fold the rpr bias during eviction on the DVE instead of
                # spending PE cycles: one add per 64-row head half
                for hh in range(2):
                    nc.vector.tensor_add(
                        dst[64 * hh:64 * hh + 64, sl],
                        ps[64 * hh:64 * hh + 64, :],
                        rpr_t[64 * b:64 * b + 64, sl])
            elif QEV_ACT:
                nc.scalar.activation(dst[:, sl], ps[:],
                                     mybir.ActivationFunctionType.Copy)
            else:
                nc.vector.tensor_copy(dst[:, sl], ps[:])

        def emit_va(b):
            vt = qkv[b][2]
            for j in range(NJ):
                tp = ps_mm.tile([128, 128], F16, tag="mm", name="tp")
                nc.tensor.transpose(tp[:], vt[:, j * 128:(j + 1) * 128], id_t[:])
                va = va_t[(b, j)]
                nc.vector.tensor_copy(
                    va[:].rearrange("p (g x) -> p g x", g=2)[:, :, 0:HD],
                    tp[:].rearrange("p (g x) -> p g x", g=2))

        def emit_proj_blocks(b, sts, yt):
            if NO_PROJ:
                return None
            it = sts[0] // 4
            if yt is None:
                yt = ysbp.tile([128, 4 * D], F16, tag="y")
            last = (sts[-1] + 1) % 4 == 0
            for stq in sts:
                ssl = slice(stq * 128, (stq + 1) * 128)
                c = stq % 4
                for eb in range(D // 512):
                    pp = ps_mm.tile([128, 512], F32, tag="mm", name="pp")
                    nc.tensor.matmul(
                        pp[:], ot2s[b][:, ssl],
                        pw_t[:, eb * 512:(eb + 1) * 512],
                        start=True, stop=True)
                    ysl = slice(c * D + eb * 512, c * D + (eb + 1) * 512)
                    if NO_YEV:
                        continue
                    if (2 * c + eb) % 4 < Y_TO_ACT:
                        nc.scalar.activation(
                            yt[:, ysl], pp[:],
                            mybir.ActivationFunctionType.Copy)
                    else:
                        nc.vector.tensor_copy(yt[:, ysl], pp[:])
            if last and not (SKIP_Y or NO_YEV):
                nc.sync.dma_start(y[b, it], yt[:])
            return yt

        def emit_att_stub(b, it):
            isl = slice(it * 512, (it + 1) * 512)
            nc.vector.tensor_copy(ot2s[b][:, isl], rpr_t[:, 0:512])
            emit_proj_blocks(b, range(4 * it, 4 * it + 4), None)

        pending_stages = []
        av_tail = []  # deferred trailing AV matmuls (TAIL_DEFER)

        def pump(n=1):
            for _ in range(n):
                if pending_stages:
                    pending_stages.pop(0)()

        def flush_finish():
            pump(len(pending_stages))

        def emit_att_it(b, it):
            if NO_ATT:
                emit_att_stub(b, it)
                return
            # scores -> exp -> AV for one 512-wide query block, software-
            # pipelined two j-chunks deep so the in-order PE queue is not
            # gated on the ACT exp latency each chunk.  The normalize +
            # projection of each block is DEFERRED into the next block's
            # scores phase so the PE never stalls on the norm chain.
            qt, kt, _ = qkv[b]
            isl = slice(it * 512, (it + 1) * 512)
            jhi = (4 * it + 3) if mode == "causal" else (NJ - 1)
            otp = [ps_ot.tile([HD + 1, 512], F32, tag="ot", name=f"ot{_h}")
                   for _h in range(HPC)]

            def emit_scores(j):
                jsl = slice(j * 128, (j + 1) * 128)
                dc = max(0, (j - 4 * it) * 128) if mode == "causal" else 0
                dct = dc if SC_TRIM else 0
                if SC_SPLIT:
                    tiles = [ps_sc.tile([128, 512], F32, tag="sc",
                                        name=f"sc{_h}") for _h in range(HPC)]
                    views = [lambda lo, hi, t=t: t[:, lo:hi] for t in tiles]
                    whole = None
                else:
                    sc2 = ps_sc.tile([128, 2 * 512], F32, tag="sc",
                                     name="sc2")
                    views = [
                        lambda lo, hi, h=h: sc2[:, h * 512 + lo:h * 512 + hi]
                        for h in range(HPC)]
                    whole = sc2
                for h in range(HPC):
                    hsl = slice(h * HD, (h + 1) * HD)
                    nc.tensor.matmul(
                        views[h](dct, 512), kt[hsl, jsl],
                        qt[hsl, isl.start + dct:isl.stop], start=True,
                        stop=True, tile_position=(h * HD, 0))
                if maskT is not None:
                    mkt = mkp.tile([128, 512], F32, tag="mk")
                    nc.sync.dma_start(mkt[:], maskT[jsl, isl])
                    for h in range(HPC):
                        nc.vector.tensor_add(
                            views[h](0, 512), views[h](0, 512), mkt[:])
                if PREMASK and mode == "causal" and j >= 4 * it:
                    # additive causal mask on the diagonal 128-block of the
                    # scores PSUM, BEFORE the exp (same semantics as the
                    # reference).  Runs ~2 j-chunks ahead of the exp, so the
                    # DVE hop is off the exp->AV critical path (unlike the
                    # 0/1-tril multiply on the exp output it replaces).
                    for h in range(HPC):
                        nc.vector.tensor_add(
                            views[h](dc, dc + 128), views[h](dc, dc + 128),
                            trimask_t)
                return views, whole

            def emit_exp_av(j, sc):
                views, whole = sc
                dc = max(0, (j - 4 * it) * 128) if mode == "causal" else 0
                diag = mode == "causal" and j >= 4 * it
                pt2 = ptp.tile([128, 2 * 512], F16, tag="pt", name="pt2")
                if HALF_EXP:
                    # timing probe: exp only head 0; head 1 reads h0's P
                    nc.scalar.activation(
                        pt2[:, dc:512], views[0](dc, 512),
                        mybir.ActivationFunctionType.Exp, scale=SCALE)
                elif dc == 0 and whole is not None and not ESPLIT:
                    nc.scalar.activation(
                        pt2[:], whole[:],
                        mybir.ActivationFunctionType.Exp, scale=SCALE)
                else:
                    # per-head exps: AV(h0) only waits on h0's half, cutting
                    # the exp->AV latency in half (ACT has slack for the
                    # extra per-instruction overhead)
                    for h in range(HPC):
                        nc.scalar.activation(
                            pt2[:, h * 512 + dc:(h + 1) * 512],
                            views[h](dc, 512),
                            mybir.ActivationFunctionType.Exp, scale=SCALE)
                va = va_t[(b, j)]

                def ptb(h):
                    return 0 if HALF_EXP else h * 512

                class _NopEng:
                    def tensor_mul(self, *a, **k):
                        pass
                eng = (_NopEng() if (NO_TRIL or PREMASK) else
                       nc.gpsimd if TRIL_ENGINE == "pool" else nc.vector)
                if diag and AV_SPLIT and not PREMASK and j > 0:
                    # the diagonal 128x128 block needs the 0/1-tril multiply
                    # on the exp output; AV over the unmasked columns runs
                    # immediately, the masked-block AV trails off-path (the
                    # accumulation order into otp does not matter)
                    for h in range(HPC):
                        if dc + 128 < 512:
                            nc.tensor.matmul(
                                otp[h][:, dc + 128:512],
                                va[:, h * (HD + 1):(h + 1) * (HD + 1)],
                                pt2[:, ptb(h) + dc + 128:ptb(h) + 512],
                                start=(j == 0), stop=(j == jhi),
                                skip_group_check=True)
                        if not (HALF_EXP and h):
                            eng.tensor_mul(
                                pt2[:, ptb(h) + dc:ptb(h) + dc + 128],
                                pt2[:, ptb(h) + dc:ptb(h) + dc + 128],
                                tril_t[:])
                        nc.tensor.matmul(
                            otp[h][:, dc:dc + 128],
                            va[:, h * (HD + 1):(h + 1) * (HD + 1)],
                            pt2[:, ptb(h) + dc:ptb(h) + dc + 128],
                            start=(j == 0), stop=(j == jhi),
                            skip_group_check=True)
                else:
                    if diag:
                        for h in range(HPC):
                            if HALF_EXP and h:
                                continue
                            eng.tensor_mul(
                                pt2[:, ptb(h) + dc:ptb(h) + dc + 128],
                                pt2[:, ptb(h) + dc:ptb(h) + dc + 128],
                                tril_t[:])
                    for h in range(1 if HALF_AV else HPC):
                        nc.tensor.matmul(
                            otp[h][:, dc:512],
                            va[:, h * (HD + 1):(h + 1) * (HD + 1)],
                            pt2[:, ptb(h) + dc:ptb(h) + 512],
                            start=(j == 0), stop=(j == jhi),
                            skip_group_check=True)

            def emit_exp_tail(j, sc):
                # exp + tril only (no AV) for a trailing chunk; the AV is
                # deferred until the NEXT block's first scores are queued,
                # so a stalled AV never gates them on the in-order PE queue
                views, whole = sc
                dc = max(0, (j - 4 * it) * 128) if mode == "causal" else 0
                diag = mode == "causal" and j >= 4 * it
                pt2 = ptp.tile([128, 2 * 512], F16, tag="pt", name="pt2")
                if dc == 0 and whole is not None:
                    nc.scalar.activation(
                        pt2[:], whole[:],
                        mybir.ActivationFunctionType.Exp, scale=SCALE)
                else:
                    for h in range(HPC):
                        nc.scalar.activation(
                            pt2[:, h * 512 + dc:(h + 1) * 512],
                            views[h](dc, 512),
                            mybir.ActivationFunctionType.Exp, scale=SCALE)
                if diag and not (NO_TRIL or PREMASK):
                    eng = (nc.gpsimd if TRIL_ENGINE == "pool" else nc.vector)
                    for h in range(HPC):
                        eng.tensor_mul(
                            pt2[:, h * 512 + dc:h * 512 + dc + 128],
                            pt2[:, h * 512 + dc:h * 512 + dc + 128],
                            tril_t[:])
                return pt2, dc

            def emit_av_tail(j, pt2, dc):
                va = va_t[(b, j)]
                for h in range(HPC):
                    nc.tensor.matmul(
                        otp[h][:, dc:512],
                        va[:, h * (HD + 1):(h + 1) * (HD + 1)],
                        pt2[:, h * 512 + dc:(h + 1) * 512],
                        start=(j == 0), stop=(j == jhi),
                        skip_group_check=True)

            pend = [(0, emit_scores(0))]
            if jhi >= 1:
                pend.append((1, emit_scores(1)))
            # previous block's deferred tail AVs go behind our first scores
            for fn in av_tail:
                fn()
            del av_tail[:]
            pump()
            for j in range(2, jhi + 1):
                pend.append((j, emit_scores(j)))
                pump()
                jd, scd = pend.pop(0)
                emit_exp_av(jd, scd)
            flush_finish()
            if TAIL_DEFER and mode == "causal" and not (HALF_EXP or HALF_AV):
                for jd, scd in pend:
                    pt2d, dcd = emit_exp_tail(jd, scd)
                    av_tail.append(
                        lambda jd=jd, p=pt2d, d=dcd: emit_av_tail(jd, p, d))
            else:
                for jd, scd in pend:
                    emit_exp_av(jd, scd)

            # the normalize + projection chain is emitted as STAGES pumped
            # one scores-chunk apart during the NEXT block, so each hop's
            # input is complete before its (in-order) engine reaches it:
            # recs -> broadcasts -> multiplies -> proj(st01) -> proj(st23).
            # The OT_aug PSUM tiles are read directly by the rec + mul
            # stages (no SBUF eviction); the ps_ot ring (bufs=2) keeps the
            # next block in the other slot, and stage_mul retires this slot
            # well before block it+2 needs it.
            st = {}

            def stage_rec():
                st["rec"] = []
                for h in range(HPC):
                    op = otp[0] if HALF_AV else otp[h]
                    dn = smp.tile([1, 512], F32, tag="dn")
                    rec = smp.tile([1, 512], F32, tag="rec")
                    if not NO_NORM:
                        # ~18-bit 1/x in a single custom-DVE op; keeps the
                        # scalar engine free for the softmax exps.  The
                        # denom row goes through SBUF first — the custom op
                        # mis-reads PSUM sources.
                        nc.vector.tensor_copy(dn[:], op[HD:HD + 1, :])
                        if NOREC:
                            nc.vector.tensor_copy(rec[:], dn[:])
                        else:
                            nc.vector.reciprocal_approx_fast(rec[:], dn[:])
                    st["rec"].append(rec)

            def stage_bc():
                st["bc"] = []
                for h in range(HPC):
                    if NORM_BC == "bcast":
                        st["bc"].append(None)
                        continue
                    bc = smp.tile([HD, 512],
                                  F16 if NORM_BC == "pe" else F32, tag="bc")
                    if not NO_NORM:
                        if NORM_BC == "pe":
                            # rank-1 outer product ones^T x rec on the PE
                            # (f32r views: 1 cyc/row); gpsimd
                            # partition_broadcast can be slow on HW
                            bcp = ps_mm.tile([HD, 512], F32, tag="mm",
                                             name="bcp")
                            nc.tensor.matmul(bcp[:],
                                             ones1f_t[:].bitcast(F32R),
                                             st["rec"][h][:].bitcast(F32R),
                                             start=True, stop=True)
                            nc.vector.tensor_copy(bc[:], bcp[:])
                        else:
                            nc.gpsimd.partition_broadcast(
                                bc[:], st["rec"][h][:])
                    st["bc"].append(bc)

            def stage_mul():
                for h in range(HPC):
                    op = otp[0] if HALF_AV else otp[h]
                    if NO_NORM:
                        src = rpr_t[0:HD, 0:512]
                    elif NORM_BC == "bcast":
                        src = st["rec"][h][:].broadcast_to([HD, 512])
                    else:
                        src = st["bc"][h][:]
                    nc.vector.tensor_mul(
                        ot2s[b][h * HD:(h + 1) * HD, isl], op[0:HD, :],
                        src)

            def stage_proj01():
                st["yt"] = emit_proj_blocks(b, range(4 * it, 4 * it + 2),
                                            None)

            def stage_proj23():
                emit_proj_blocks(b, range(4 * it + 2, 4 * it + 4), st["yt"])

            if INLINE_NORM:
                # rec/bc/mul touch only DVE+Pool; emitting them inline at
                # block end starts the chain earlier and frees the ot PSUM
                # ring sooner.  Only the PE-bound proj stages stay deferred.
                stage_rec()
                stage_bc()
                stage_mul()
                pending_stages.extend(
                    [lambda: None, stage_proj01, stage_proj23])
            else:
                pending_stages.extend(
                    [stage_rec, stage_bc, stage_mul, stage_proj01,
                     stage_proj23])

        def emit_va_set(b, sb):
            # V^T -> va transposes for j-chunks 4sb..4sb+3 (follows the
            # (2, sb) projection group that produced those vt columns)
            vt = qkv[b][2]
            for j in range(4 * sb, 4 * sb + 4):
                va = va_t[(b, j)]
                if VA_DMA:
                    # DMA xbar transpose to a 2D scratch (the xbar can't
                    # target the interleaved va view), then DVE-interleave;
                    # saves the PE pass + PSUM ring slot
                    tps = smp.tile([128, 128], F16, tag="vtp", name="vtp")
                    nc.sync.dma_start_transpose(
                        tps[:], vt[:, j * 128:(j + 1) * 128])
                    nc.vector.tensor_copy(
                        va[:].rearrange("p (g x) -> p g x", g=2)[:, :, 0:HD],
                        tps[:].rearrange("p (g x) -> p g x", g=2))
                    continue
                tp = ps_mm.tile([128, 128], F16, tag="mm", name="tp")
                nc.tensor.transpose(tp[:], vt[:, j * 128:(j + 1) * 128],
                                    id_t[:])
                nc.vector.tensor_copy(
                    va[:].rearrange("p (g x) -> p g x", g=2)[:, :, 0:HD],
                    tp[:].rearrange("p (g x) -> p g x", g=2))

        def fill_units(b):
            # filler work in next-phase consumption order: (q,k,v,va) per
            # 512-col block, so att(b, it) of the next phase finds qt/kt/va
            # for its own window already complete, and the va copies are
            # spread across the phase instead of bursting at its end
            units = []
            for sb in range(NI):
                for p in range(3):
                    units.append(
                        lambda b=b, p=p, sb=sb: emit_qkv_psb(b, p, sb))
                units.append(lambda b=b, sb=sb: emit_va_set(b, sb))
            return units

        def prologue():
            # one-time pipeline fill: batch 0's QKV ahead of the loop, and
            # batch 1's x staged so the first body's fillers don't wait.
            # loads(1) is issued AFTER the QKV emission: the 8 DMA queues
            # share bandwidth, so batching both 4MB loads up front would
            # delay batch 0's first chunk (and the whole pipeline) 2x.
            emit_loads(0)
            for g in range(3 * NI):
                emit_qkv_group(0, g)
            emit_loads(1)
            emit_va(0)

        def body(_iv=None, last=False):
            # Software-pipelined across iterations: each batch's attention
            # overlaps the OTHER batch's QKV/va filler units on the in-order
            # PE queue; batch 0's QKV belongs to the NEXT iteration (filled
            # by the prologue for the first one).  x loads are issued a full
            # phase ahead of their consuming fillers so chunk arrival never
            # stalls the PE queue.
            if SCHED == "v2":
                gs, ge = [0, 0, 2, 6], [0, 2, 6, 12]
                emit_loads(1)
                for it in range(NI):
                    emit_att_it(0, it)
                    for g in range(gs[it], ge[it]):
                        emit_qkv_group(1, g)
                emit_va(1)
                emit_loads(0)
                for it in range(NI):
                    emit_att_it(1, it)
                    for g in range(gs[it], ge[it]):
                        emit_qkv_group(0, g)
                emit_va(0)
                for fn in av_tail:
                    fn()
                del av_tail[:]
                flush_finish()
                return
            units1 = fill_units(1)
            if not last:
                # x loads and batch-0 fillers feed the NEXT iteration; on
                # the final (or only) iteration they are dead work inside
                # the measured NEFF — skip them
                emit_loads(0)
            for it in range(NI):
                emit_att_it(0, it)
                for u in range(UB[it], UB[it + 1]):
                    units1[u]()
            units0 = fill_units(0)
            if not last:
                emit_loads(1)
            for it in range(NI):
                emit_att_it(1, it)
                if not last:
                    for u in range(UB[it], UB[it + 1]):
                        units0[u]()
            for fn in av_tail:
                fn()
            del av_tail[:]
            flush_finish()

        return prologue, body

    nc._dbg = {"qkv": qkv, "ot2s": ot2s, "va": va_t, "xts": xts}
    prologue, body = make_body()
    prologue()
    if niter >= 1:
        for i in range(niter):
            body(last=(i == niter - 1))
    else:
        with tc.For_i(0, -niter, 1) as iv:
            body(iv)

    for c in reversed(ctxs):
        c.__exit__(None, None, None)


def _build(mode, niter=1):
    key = (mode, niter)
    if key in _BUILD_CACHE:
        return _BUILD_CACHE[key]
    nc = bacc.Bacc("TRN2", target_bir_lowering=False, debug=False,
                   num_devices=N_CORES)
    t = {}
    t["xt"] = nc.dram_tensor("xt", (B, 128, KC * S), F16, kind="ExternalInput")
    t["wall"] = nc.dram_tensor("wall", (128, 3 * KC * DL), F16,
                               kind="ExternalInput")
    t["pwT"] = nc.dram_tensor("pwT", (DL, D), F16, kind="ExternalInput")
    t["rpr2T"] = nc.dram_tensor("rpr2T", (128, S), F16, kind="ExternalInput")
    t["misc"] = nc.dram_tensor("misc", (128, 4 * 128 + 1 + HD), F16,
                               kind="ExternalInput")
    if mode == "generic":
        t["maskT"] = nc.dram_tensor("maskT", (S, S), F32, kind="ExternalInput")
    t["y"] = nc.dram_tensor("y", (B, NI, 128, 4 * D), F16,
                            kind="ExternalOutput")

    with tile.TileContext(nc) as tc, \
            nc.allow_low_precision(reason="fp16 matmul operands"):
        _emit(nc, tc, t, mode, niter)
    nc.compile()
    _BUILD_CACHE[key] = (nc, t)
    return nc, t


def _prep_inputs(x, positions, causal_mask, wq, wk, wv, rpr, proj_w):
    """Host-side shard prep.  Returns (mode, per-core input maps)."""
    mask = np.asarray(causal_mask, np.float32).reshape(S, S)
    low = np.tril(np.ones((S, S), dtype=bool))
    if (mask[low] == 0.0).all() and (mask.any() and
                                     np.all(mask[~low] <= -1e6)):
        mode = "causal"
    elif not mask.any():
        mode = "zero"
    else:
        mode = "generic"

    # xt layout: (B, 128, KC*S): [b, p, k*S + s] = x[b, s, k*128 + p]
    xt = np.asarray(x, np.float32).transpose(0, 2, 1).reshape(B, KC, 128, S)
    xt = np.ascontiguousarray(xt.transpose(0, 2, 1, 3)).reshape(
        B, 128, KC * S).astype(np.float16)
    pos = np.asarray(positions).astype(np.int64)
    rpr_g = np.asarray(rpr, np.float32)[pos]  # (B, S, HD)
    rpr2 = np.ascontiguousarray(
        rpr_g.transpose(0, 2, 1)).reshape(B * HD, S).astype(np.float16)
    jj = np.arange(128)[:, None]
    ii = np.arange(128)[None, :]
    tril01 = (jj <= ii).astype(np.float16)
    ident = np.eye(128, dtype=np.float16)
    i2h = np.concatenate([np.eye(64), np.eye(64)], axis=1)
    i2 = np.concatenate([i2h, i2h], axis=0).astype(np.float16)
    trimask = ((jj > ii) * np.float32(-60000.0)).astype(np.float16)
    misc = np.concatenate(
        [tril01, ident, i2, np.ones((128, 1 + HD), np.float16), trimask],
        axis=1).astype(np.float16)
    maskT = np.ascontiguousarray(mask.T) if mode == "generic" else None

    wq = np.asarray(wq, np.float32)
    wk = np.asarray(wk, np.float32)
    wv = np.asarray(wv, np.float32)
    pw = np.asarray(proj_w, np.float32)

    def wall_of(wT):
        # [D, DL] -> [128, KC*DL]: col (k*DL + c), row r = wT[k*128 + r, c]
        return wT.reshape(KC, 128, DL).transpose(1, 0, 2).reshape(128,
                                                                  KC * DL)

    in_maps = []
    for c in range(N_CORES):
        rs = slice(c * DL, (c + 1) * DL)
        wall = np.concatenate(
            [wall_of(np.ascontiguousarray(w[rs, :].T))
             for w in (wq, wk, wv)], axis=1).astype(np.float16)
        m = {
            "xt": xt,
            "wall": np.ascontiguousarray(wall),
            "pwT": np.ascontiguousarray(pw[:, rs].T).astype(np.float16),
            "rpr2T": rpr2,
            "misc": misc,
        }
        if maskT is not None:
            m["maskT"] = maskT
        in_maps.append(m)
    return mode, in_maps


def kernel(x, positions, causal_mask, wq, wk, wv, rpr, proj_w, proj_b,
           _niter=1, **_ignored):
    mode, in_maps = _prep_inputs(x, positions, causal_mask, wq, wk, wv, rpr,
                                 proj_w)
    nc, _ = _build(mode, _niter)
    res = run_bass_kernel_spmd(nc, in_maps, core_ids=list(range(N_CORES)))
    out = np.zeros((B, S, D), dtype=np.float32)
    for r in res.results:
        # y layout: (B, NI, 128, 4*1024): [b, it, s, c*1024 + d]
        yr = r["y"].astype(np.float32).reshape(B, NI, 128, 4, D)
        out += yr.transpose(0, 1, 3, 2, 4).reshape(B, S, D)
    out += np.asarray(proj_b, np.float32)[None, None, :]
    return out

